# revision 1
# baseline (speedup 1.0000x reference)
"""Causal single-head attention (B=4, T=4096, E=1024, H=64) on 8 TRN2 cores.

Sharding: 2 cores per batch; no collectives (host shards, device computes,
host gathers). Queries are assigned to cores in 256-row half-groups with the
fold pattern {0,3}/{1,2} (mod 4), which makes both cores' causal work-lists
IDENTICAL: 8 query slots with key-group trip counts exactly (1..8), so one
SPMD graph serves all cores; all per-core variation (which queries, causal
mask content, key order) lives in host-prepared input data.

Host prep (layout-only, no FLOPs): x[b]^T cast to bf16 with columns permuted
to [owned half-groups in slot order | partner half-groups in the other
core's slot order]. Because each original 512-token key-pair always splits
one-owned/one-foreign, the compiled per-key-group column offsets are
identical on every core; the device re-gathers each key-group as
[own-half | foreign-half], and causal masks (shipped as input data) encode
the per-core key order.

Per-core device graph (bf16 compute, f32 PSUM; key-group-major, PE
software-pipelined so exp(i) overlaps scores(i+1)):
  - Q^T projection (owned tokens, slot order); K^T/V^T projection with
    lhsT=[Wk|Wv] for full 128-wide PE utilization, interleaved with
    attention as each key-group's data lands (DMA/compute overlap)
  - V^T -> V1 [128-token blocks, 65] via PE transpose; column 64 = ones so
    the softmax denominator falls out of the attn@V matmul
  - Adjacent slot pairs share N=512 score/AV matmuls (fewer LDWEIGHTS);
    exp on ACT with scale=E^-0.5 folded in; multiplicative causal mask (DVE)
    on diagonal items only; U^T[65,:] += V1_blk.T @ exp^T accumulated in
    PSUM per item, summed across key-groups in SBUF (DVE)
  - Epilogue per slot: PE-transpose U^T, DVE reciprocal of the denominator
    column, per-partition scale, DMA out f32.
"""
import numpy as np
import ml_dtypes

B, T, E, H = 4, 4096, 1024, 64
HGS = 256         # queries per slot (half-group size)
KG = 512          # keys per key-group
NSLOT = 8
NQ = NSLOT * HGS  # 2048 owned queries per core
ET = E // 128     # 8 E-tiles
NKB = T // 128    # 32 key blocks
SCALE = float(E) ** -0.5

HGS_A = [0, 3, 4, 7, 8, 11, 12, 15]   # core half 0: needs 1..8 in slot order
HGS_B = [1, 2, 5, 6, 9, 10, 13, 14]   # core half 1: needs 1..8 in slot order

_cache = {}


def _bf16(a):
    return np.ascontiguousarray(a.astype(ml_dtypes.bfloat16))


def _build_graph():
    import concourse.mybir as mybir
    import concourse.tile as tile
    from concourse import bacc
    from concourse.masks import make_identity

    dt = mybir.dt
    nc = bacc.Bacc(None, target_bir_lowering=False)
    xT_e = nc.declare_dram_parameter("xT", [E, T], dt.bfloat16, isOutput=False)
    wkv_e = nc.declare_dram_parameter("wkv", [E, 128], dt.bfloat16, isOutput=False)
    wq_e = nc.declare_dram_parameter("wq", [E, H], dt.bfloat16, isOutput=False)
    mask_e = nc.declare_dram_parameter("mask", [128, NSLOT * 4 * HGS], dt.bfloat16,
                                       isOutput=False)
    out_e = nc.declare_dram_parameter("out", [NQ, H], dt.float32, isOutput=True)

    xT_r = xT_e.rearrange("(et p) t -> p et t", p=128)

    with tile.TileContext(nc) as tc:
        with (
            tc.tile_pool(name="singles", bufs=1) as singles,
            tc.tile_pool(name="persist", bufs=1) as persist,
        ):
            identity = singles.tile([128, 128], dt.bfloat16)
            make_identity(nc, identity)
            identity32 = singles.tile([H + 1, H + 1], dt.float32)
            make_identity(nc, identity32)
            wkv_sb = singles.tile([128, ET, 128], dt.bfloat16)
            nc.scalar.dma_start(out=wkv_sb, in_=wkv_e.rearrange("(et p) m -> p et m", p=128))
            wq_sb = singles.tile([128, ET, H], dt.bfloat16)
            nc.scalar.dma_start(out=wq_sb, in_=wq_e.rearrange("(et p) m -> p et m", p=128))
            mask_sb = singles.tile([128, NSLOT * 4 * HGS], dt.bfloat16)

            # persistent activations
            kvT = persist.tile([128, T], dt.bfloat16)    # rows 0:64 K^T, 64:128 V^T
            v1 = persist.tile([128, NKB, H + 1], dt.bfloat16)
            qT = persist.tile([64, NQ], dt.bfloat16)
            u_acc = persist.tile([H + 1, NSLOT, HGS], dt.float32)
            # per pair g: columns [own_2g | foreign_2g | own_2g+1 | foreign_2g+1]
            xq_tiles = [persist.tile([128, ET, 4, HGS], dt.bfloat16,
                                     name=f"xq{g}") for g in range(4)]

            nc.vector.memset(v1[:, :, H], 1.0)  # denominator ones column

            # ---- fused pipeline: proj + attention, key-group-major ----
            with (
                tc.tile_pool(name="pscore", bufs=2, space="PSUM") as pscore,
                tc.tile_pool(name="pproj", bufs=2, space="PSUM") as pproj,
                tc.tile_pool(name="pu", bufs=1, space="PSUM") as pu,
                tc.tile_pool(name="pepi", bufs=1, space="PSUM") as pepi,
                tc.tile_pool(name="ex", bufs=3) as expool,
                tc.tile_pool(name="epi", bufs=4) as epi,
            ):
                def prefetch_kv(s):
                    if s < T // KG:
                        nc.sync.dma_start(
                            out=xq_tiles[s // 2][:, :, 2 * (s % 2) + 1, :],
                            in_=xT_r[:, :, NQ + s * HGS:NQ + (s + 1) * HGS])

                def qdma(g):
                    for two in range(2):
                        nc.sync.dma_start(
                            out=xq_tiles[g][:, :, 2 * two, :],
                            in_=xT_r[:, :, g * KG + two * HGS:
                                     g * KG + (two + 1) * HGS])
                    return g

                def qmm(g, _xg=None):
                    own = xq_tiles[g].rearrange(
                        "p et (two fo) c -> p et two fo c", fo=2)[:, :, :, 0, :]
                    ps = pproj.tile([128, KG], dt.float32, tag="pj", name="ps_q")
                    if g == 0:
                        # group 0 gates the first attention item: use the
                        # shortest-latency single chain + direct copy
                        for et in range(ET):
                            nc.tensor.matmul(ps[0:64, :], lhsT=wq_sb[:, et, :],
                                             rhs=own[:, et, :, :],
                                             start=(et == 0),
                                             stop=(et == ET - 1))
                        nc.scalar.copy(out=qT[:, 0:KG], in_=ps[0:64, :])
                        return
                    # groups 1-3: two concurrent M=64 chains on PE column
                    # groups (0,0)/(0,64) (even/odd E-tiles), summed on DVE —
                    # ~2x PE throughput via tile_position concurrency.
                    for et in range(ET):
                        col = 64 * (et % 2)
                        nc.tensor.matmul(ps[col:col + 64, :],
                                         lhsT=wq_sb[:, et, :],
                                         rhs=own[:, et, :, :],
                                         tile_position=(0, col),
                                         start=(et < 2), stop=(et >= ET - 2),
                                         skip_group_check=True)
                    qa = epi.tile([64, KG], dt.float32, tag="qa")
                    qb = epi.tile([64, KG], dt.float32, tag="qb")
                    nc.scalar.copy(out=qa, in_=ps[0:64, :])
                    nc.vector.tensor_copy(out=qb, in_=ps[64:128, :])
                    nc.vector.tensor_add(qT[:, g * KG:(g + 1) * KG], qa, qb)

                if True:
                    # pending: (q0 col, width, j, exT, done_slots)
                    pending = []
                    last_exp = [None]

                    def pitem_front(p, j):
                        """Paired item: slots (2p, 2p+1), key-group j, N=512.
                        Masked on slot 2p's half when j == 2p (its diagonal)."""
                        a = 2 * p
                        q_ap = qT[:, a * HGS:(a + 2) * HGS]
                        exT = expool.tile([128, 4, 2 * HGS], dt.bfloat16, tag="ex")
                        for half in range(2):
                            psh = pscore.tile([128, 2, 2 * HGS], dt.float32, tag="sc",
                                              name="ps_h")
                            for rr in range(2):
                                r = 2 * half + rr
                                kb = 4 * j + r
                                nc.tensor.matmul(
                                    psh[:, rr, :],
                                    lhsT=kvT[0:64, kb * 128:(kb + 1) * 128],
                                    rhs=q_ap, start=True, stop=True)
                            last_exp[0] = nc.scalar.activation(
                                out=exT[:, 2 * half:2 * half + 2, :], in_=psh,
                                func=mybir.ActivationFunctionType.Exp, scale=SCALE)
                        if j == a:
                            nc.vector.tensor_mul(
                                exT[:, :, 0:HGS], exT[:, :, 0:HGS],
                                mask_sb[:, a * 4 * HGS:(a + 1) * 4 * HGS]
                                .rearrange("p (r c) -> p r c", r=4))
                        done = [a] if j == a else []
                        pending.append((a, 2, j, exT, done))

                    def sitem_front(b):
                        """Solo diagonal item for odd slot b at key-group j=b."""
                        j = b
                        q_ap = qT[:, b * HGS:(b + 1) * HGS]
                        exT = expool.tile([128, 4, HGS], dt.bfloat16, tag="ex",
                                          name="exs")
                        ps4 = pscore.tile([128, 4, HGS], dt.float32, tag="sc",
                                          name="ps_s")
                        for r in range(4):
                            kb = 4 * j + r
                            nc.tensor.matmul(
                                ps4[:, r, :],
                                lhsT=kvT[0:64, kb * 128:(kb + 1) * 128],
                                rhs=q_ap, start=True, stop=True)
                        nc.scalar.activation(
                            out=exT, in_=ps4,
                            func=mybir.ActivationFunctionType.Exp, scale=SCALE)
                        nc.vector.tensor_mul(
                            exT, exT,
                            mask_sb[:, b * 4 * HGS:(b + 1) * 4 * HGS]
                            .rearrange("p (r c) -> p r c", r=4))
                        pending.append((b, 1, j, exT, [b]))

                    def flush_av():
                        s0, w, j, exT, done = pending.pop(0)
                        u_it = pu.tile([H + 1, 2 * HGS], dt.float32, tag="u")
                        uv = u_it[:, 0:w * HGS]
                        for r in range(4):
                            nc.tensor.matmul(
                                uv, lhsT=v1[:, 4 * j + r, :],
                                rhs=exT[:, r, :],
                                start=(r == 0), stop=(r == 3))
                        acc = u_acc[:, s0, :] if w == 1 else \
                            u_acc[:, s0:s0 + 2, :].rearrange("p a c -> p (a c)")
                        if j == 0:
                            nc.vector.tensor_copy(out=acc, in_=uv)
                        else:
                            nc.vector.tensor_add(acc, acc, uv)
                        for s in done:
                            epilogue(s)

                    def epilogue(s):
                        for hh in range(2):
                            pst = pepi.tile([128, H + 1], dt.float32, tag="tp")
                            nc.tensor.transpose(
                                pst, u_acc[:, s, hh * 128:(hh + 1) * 128],
                                identity32[:, :])
                            rec = epi.tile([128, 1], dt.float32, tag="rec")
                            nc.vector.reciprocal(rec, pst[:, H:H + 1])
                            o_sb = epi.tile([128, H], dt.float32, tag="o")
                            nc.vector.tensor_scalar_mul(o_sb, pst[:, 0:H], rec)
                            row0 = s * HGS + hh * 128
                            nc.scalar.dma_start(out=out_e[row0:row0 + 128, :],
                                                in_=o_sb)

                    def kvproj(j, split=False):
                        xj = xq_tiles[j // 2][:, :, 2 * (j % 2):2 * (j % 2) + 2, :]
                        psp = pproj.tile([128, KG], dt.float32, tag="pj")
                        if split:
                            for two in range(2):
                                for et in range(ET):
                                    nc.tensor.matmul(
                                        psp[:, two * HGS:(two + 1) * HGS],
                                        lhsT=wkv_sb[:, et, :],
                                        rhs=xj[:, et, two, :],
                                        start=(et == 0), stop=(et == ET - 1))
                        else:
                            for et in range(ET):
                                nc.tensor.matmul(
                                    psp, lhsT=wkv_sb[:, et, :],
                                    rhs=xj[:, et, :, :],
                                    start=(et == 0), stop=(et == ET - 1))
                        nc.vector.tensor_copy(out=kvT[:, j * KG:(j + 1) * KG],
                                              in_=psp)
                        for r in range(4):
                            kb = 4 * j + r
                            pst = pepi.tile([128, H + 1], dt.bfloat16, tag="tp",
                                            name="pst_vt")
                            nc.tensor.transpose(
                                pst[:, 0:H], kvT[64:128, kb * 128:(kb + 1) * 128],
                                identity[64:128, 64:128])
                            nc.vector.tensor_copy(out=v1[:, kb, 0:H], in_=pst[:, 0:H])

                    def pitem(p, j):
                        pitem_front(p, j)
                        while len(pending) > 1:
                            flush_av()

                    def sitem(b):
                        sitem_front(b)
                        while len(pending) > 1:
                            flush_av()

                    # step 0: interleave Q projection groups with step-0 items.
                    # DMA ring order: xT0, xq0, mask01, xq1, xT1, xq2, xq3,
                    # xT2 ... so no consumer waits.
                    prefetch_kv(0)
                    xq_t = {0: qdma(0)}
                    kvproj(0)
                    qmm(0, xq_t.pop(0))
                    nc.scalar.dma_start(out=mask_sb[:, 0:2 * 4 * HGS],
                                        in_=mask_e[:, 0:2 * 4 * HGS])
                    xq_t[1] = qdma(1)
                    pitem(0, 0)       # masked on slot 0 (its diagonal)
                    xq_t[2] = qdma(2)
                    xq_t[3] = qdma(3)
                    prefetch_kv(1)
                    qmm(1, xq_t.pop(1))
                    pitem(1, 0)
                    qmm(2, xq_t.pop(2))
                    prefetch_kv(2)
                    pitem(2, 0)
                    qmm(3, xq_t.pop(3))
                    pitem(3, 0)
                    # steps 1..7: paired items for pairs p >= ceil(j/2); odd j
                    # additionally has the solo diagonal of slot j
                    for j in range(1, NSLOT):
                        kvproj(j)
                        if j == 1:
                            m2 = nc.scalar.dma_start(out=mask_sb[:, 2 * 4 * HGS:],
                                                     in_=mask_e[:, 2 * 4 * HGS:])
                            tile.add_dep_helper(last_exp[0].ins, m2.ins,
                                                sync=False,
                                                reason="mask2 after step0 exps")
                        prefetch_kv(j + 2)
                        for p in range((j + 1) // 2, 4):
                            pitem(p, j)
                        if j % 2 == 1:
                            sitem(j)  # diag of odd slot j, emitted last
                    while pending:
                        flush_av()
    nc.compile()
    return nc


def _make_masks(hgs):
    """Diagonal masks for the per-core key order [own hg | partner hg] within
    each key-group: rows 0:256 self-triangle, rows 256:512 all-valid iff the
    own half-group is the later (odd) member of its pair."""
    masks = np.zeros((NSLOT, KG, HGS), dtype=np.float32)
    rk = np.arange(HGS)[:, None]
    cq = np.arange(HGS)[None, :]
    for s, hg in enumerate(hgs):
        masks[s, 0:HGS, :] = (rk <= cq)
        masks[s, HGS:KG, :] = 1.0 if hg % 2 == 1 else 0.0
    # device layout: [partition 128, slot, blockrow 4, col 256]
    m = masks.reshape(NSLOT, 4, 128, HGS).transpose(2, 0, 1, 3)
    return _bf16(m.reshape(128, NSLOT * 4 * HGS))


def kernel(x, Wk, Wq, Wv):
    from concourse.bass_utils import run_bass_kernel_spmd

    x = np.asarray(x, dtype=np.float32)
    Wk = np.asarray(Wk, dtype=np.float32)
    Wq = np.asarray(Wq, dtype=np.float32)
    Wv = np.asarray(Wv, dtype=np.float32)

    if "nc" not in _cache:
        _cache["nc"] = _build_graph()
    nc = _cache["nc"]

    wkv = _bf16(np.concatenate([Wk, Wv], axis=1))
    wq = _bf16(Wq)
    mask_by_half = [_make_masks(HGS_A), _make_masks(HGS_B)]

    in_maps = []
    core_meta = []
    for b in range(B):
        xTb = _bf16(x[b].T)  # [E, T]
        for half, hgs in enumerate([HGS_A, HGS_B]):
            other = [HGS_A, HGS_B][1 - half]
            xp = np.concatenate(
                [xTb[:, hg * HGS:(hg + 1) * HGS] for hg in list(hgs) + other],
                axis=1)
            in_maps.append({
                "xT": np.ascontiguousarray(xp),
                "wkv": wkv,
                "wq": wq,
                "mask": mask_by_half[half],
            })
            core_meta.append((b, hgs))

    res = run_bass_kernel_spmd(nc, in_maps, core_ids=list(range(8)),
                               **_cache.get("run_kwargs", {}))
    _cache["last_result"] = res

    full = np.zeros((B, T, H), dtype=np.float32)
    for core, (b, hgs) in enumerate(core_meta):
        o = res.results[core]["out"]
        for s, hg in enumerate(hgs):
            full[b, hg * HGS:(hg + 1) * HGS, :] = o[s * HGS:(s + 1) * HGS, :]
    return full



# revision 10
# speedup vs baseline: 1.1012x; 1.1012x over previous
"""Causal single-head attention (B=4, T=4096, E=1024, H=64) on 8 TRN2 cores.

Sharding: 2 cores per batch; no collectives (host shards, device computes,
host gathers). Queries are assigned to cores in 256-row half-groups with the
fold pattern {0,3}/{1,2} (mod 4), which makes both cores' causal work-lists
IDENTICAL: 8 query slots with key-group trip counts exactly (1..8), so one
SPMD graph serves all cores; all per-core variation (which queries, causal
mask content, key order) lives in host-prepared input data.

Host prep (layout-only, no FLOPs): x[b]^T cast to bf16 with columns permuted
to [owned half-groups in slot order | partner half-groups in the other
core's slot order]. Slot j's 256 queries are exactly the own half of
key-group j, so one fused [Wk|Wq] projection pass over the own columns
yields both K^T and Q^T; V1 for own tokens is computed directly
(lhsT=x-block, rhs=Wv) with full 128-partition output. Foreign columns get
a [Wk|Wv] pass + PE transposes for V1. The diagonal causal mask is a single
shared 256x256 triangle (identical for every slot and core) plus a per-slot
0/1 parity scalar for the foreign half -- tiny inputs instead of per-slot
masks.

Device (bf16 compute, f32 PSUM): items (pair p, key-group j) stream
pair-major; each pair's attn@V accumulates IN PSUM across its whole item
stream (matmul start/stop flags span items), so there are no per-item
accumulate ops at all. exp on ACT with scale=E^-0.5 folded in; V1 carries a
ones column so the softmax denominator falls out of the attn@V matmul.
Epilogue per pair: PSUM->SBUF copy, PE-transpose, reciprocal * scale, DMA.
Input DMAs ride SP in feed order; PE warms up on identity matmuls while the
first DMAs land.
"""
import numpy as np
import ml_dtypes

B, T, E, H = 4, 4096, 1024, 64
HGS = 256         # queries per slot (half-group size)
KG = 512          # keys per key-group
NSLOT = 8
NQ = NSLOT * HGS  # 2048 owned queries per core
ET = E // 128     # 8 E-tiles
NKB = T // 128    # 32 key blocks
SCALE = float(E) ** -0.5
N_WARM = 26       # PE warmup matmuls (N=128) while first DMAs land

HGS_A = [0, 3, 4, 7, 8, 11, 12, 15]   # core half 0: needs 1..8 in slot order
HGS_B = [1, 2, 5, 6, 9, 10, 13, 14]   # core half 1: needs 1..8 in slot order

_cache = {}


def _bf16(a):
    return np.ascontiguousarray(a.astype(ml_dtypes.bfloat16))


def _build_graph():
    import concourse.mybir as mybir
    import concourse.tile as tile
    from concourse import bacc
    from concourse.masks import make_identity

    dt = mybir.dt
    nc = bacc.Bacc(None, target_bir_lowering=False)
    xT_e = nc.declare_dram_parameter("xT", [E, T], dt.bfloat16, isOutput=False)
    wkv_e = nc.declare_dram_parameter("wkv", [E, 128], dt.bfloat16, isOutput=False)
    wkq_e = nc.declare_dram_parameter("wkq", [E, 128], dt.bfloat16, isOutput=False)
    tri_e = nc.declare_dram_parameter("tri", [128, 2 * HGS], dt.bfloat16,
                                      isOutput=False)
    dsel_e = nc.declare_dram_parameter("dsel", [128, NSLOT], dt.float32,
                                       isOutput=False)
    out_e = nc.declare_dram_parameter("out", [NQ, H], dt.float32, isOutput=True)

    xT_r = xT_e.rearrange("(et p) t -> p et t", p=128)

    with tile.TileContext(nc) as tc:
        with (
            tc.tile_pool(name="singles", bufs=1) as singles,
            tc.tile_pool(name="persist", bufs=1) as persist,
        ):
            identity = singles.tile([128, 128], dt.bfloat16)
            make_identity(nc, identity)
            identity32 = singles.tile([H + 1, H + 1], dt.float32)
            make_identity(nc, identity32)
            wkv_sb = singles.tile([128, ET, 128], dt.bfloat16)
            wkq_sb = singles.tile([128, ET, 128], dt.bfloat16)
            tri_sb = singles.tile([128, 2, HGS], dt.bfloat16)
            dsel_sb = singles.tile([128, NSLOT], dt.float32)

            # persistent activations
            kvT = persist.tile([128, T], dt.bfloat16)    # 0:64 K^T, 64:128 V^T(frn)
            qT = persist.tile([64, NQ], dt.bfloat16)
            v1 = persist.tile([128, NKB, H + 1], dt.bfloat16)
            # per pair g: columns [own_2g | foreign_2g | own_2g+1 | foreign_2g+1]
            xq_tiles = [persist.tile([128, ET, 4, HGS], dt.bfloat16,
                                     name=f"xq{g}") for g in range(4)]

            nc.vector.memset(v1[:, :, H], 1.0)  # denominator ones column

            with (
                tc.tile_pool(name="pscore", bufs=3, space="PSUM") as pscore,
                tc.tile_pool(name="paux", bufs=2, space="PSUM") as paux,
                tc.tile_pool(name="pu", bufs=1, space="PSUM") as pu,
                tc.tile_pool(name="ex", bufs=3) as expool,
                tc.tile_pool(name="epi", bufs=4) as epi,
            ):
                # ---- DMA issue helpers (all inputs on SP, feed order) ----
                def dma_w():
                    nc.sync.dma_start(
                        out=wkq_sb, in_=wkq_e.rearrange("(et p) m -> p et m", p=128))

                def dma_w2():
                    nc.sync.dma_start(
                        out=wkv_sb, in_=wkv_e.rearrange("(et p) m -> p et m", p=128))
                    nc.sync.dma_start(out=tri_sb,
                                      in_=tri_e.rearrange("p (r c) -> p r c", r=2))
                    nc.sync.dma_start(out=dsel_sb, in_=dsel_e[:, :])

                def qdma(g, two):
                    # own half for key-group 2g+two -> c-slot 2*two
                    nc.sync.dma_start(
                        out=xq_tiles[g][:, :, 2 * two, :],
                        in_=xT_r[:, :, g * KG + two * HGS:g * KG + (two + 1) * HGS])

                def fdma(j):
                    # foreign half for key-group j -> c-slot 2*(j%2)+1
                    nc.sync.dma_start(
                        out=xq_tiles[j // 2][:, :, 2 * (j % 2) + 1, :],
                        in_=xT_r[:, :, NQ + j * HGS:NQ + (j + 1) * HGS])

                # ---- projection passes ----
                def own(j):
                    """[Wk|Wq] over own cols of key-group j: K^T own half +
                    Q^T of slot j (slot j's queries ARE its own keys)."""
                    xo = xq_tiles[j // 2][:, :, 2 * (j % 2), :]
                    ps = paux.tile([128, HGS], dt.float32, tag="a")
                    for et in range(ET):
                        nc.tensor.matmul(ps, lhsT=wkq_sb[:, et, :],
                                         rhs=xo[:, et, :],
                                         start=(et == 0), stop=(et == ET - 1))
                    nc.vector.tensor_copy(out=kvT[0:64, j * KG:j * KG + HGS],
                                          in_=ps[0:64, :])
                    nc.vector.tensor_copy(out=qT[:, j * HGS:(j + 1) * HGS],
                                          in_=ps[64:128, :])

                def v1own(j):
                    """V1 for own tokens of key-group j, directly:
                    out[tok,H] = sum_et x_blk^T.T @ Wv_et (full-M, free=64)."""
                    xo = xq_tiles[j // 2][:, :, 2 * (j % 2), :]
                    for b in range(2):
                        psv = paux.tile([128, H], dt.float32, tag="a", name="psv")
                        for et in range(ET):
                            nc.tensor.matmul(
                                psv, lhsT=xo[:, et, b * 128:(b + 1) * 128],
                                rhs=wkv_sb[:, et, 64:128],
                                start=(et == 0), stop=(et == ET - 1))
                        nc.vector.tensor_copy(out=v1[:, 4 * j + b, 0:H], in_=psv)

                def foreign(j):
                    """[Wk|Wv] over foreign cols of key-group j, then PE
                    transposes of V^T into V1 blocks 4j+2, 4j+3."""
                    xf = xq_tiles[j // 2][:, :, 2 * (j % 2) + 1, :]
                    ps = paux.tile([128, HGS], dt.float32, tag="a")
                    for et in range(ET):
                        nc.tensor.matmul(ps, lhsT=wkv_sb[:, et, :],
                                         rhs=xf[:, et, :],
                                         start=(et == 0), stop=(et == ET - 1))
                    nc.vector.tensor_copy(
                        out=kvT[:, j * KG + HGS:(j + 1) * KG], in_=ps)
                    for b in range(2):
                        kb = 4 * j + 2 + b
                        pst = paux.tile([128, H], dt.bfloat16, tag="a",
                                       name="pst_vt")
                        nc.tensor.transpose(
                            pst, kvT[64:128, kb * 128:(kb + 1) * 128],
                            identity[64:128, 64:128])
                        nc.vector.tensor_copy(out=v1[:, kb, 0:H], in_=pst)

                # ---- attention items, pair-major with PSUM-resident acc ----
                # pending: (exT, j, width, uv, av_start, av_stop, diag_slot)
                pending = []

                def flush_av():
                    exT, j, w, uv, av_start, av_stop, _ = pending.pop(0)
                    # solo items (w=1) cover only the odd slot's column half
                    uvs = uv if w == 2 else uv[:, HGS:2 * HGS]
                    for r in range(4):
                        nc.tensor.matmul(
                            uvs, lhsT=v1[:, 4 * j + r, :], rhs=exT[:, r, :],
                            start=(av_start and r == 0),
                            stop=(av_stop and r == 3),
                            skip_group_check=True)

                def pitem_front(p, j, uvp, av_start, av_stop):
                    """Paired item: slots (2p, 2p+1), key-group j, N=512.
                    Diagonal-masked on slot 2p's half when j == 2p."""
                    a = 2 * p
                    q_ap = qT[:, a * HGS:(a + 2) * HGS]
                    exT = expool.tile([128, 4, 2 * HGS], dt.bfloat16, tag="ex")
                    for half in range(2):
                        psh = pscore.tile([128, 2, 2 * HGS], dt.float32,
                                          tag="sc", name="ps_h")
                        for rr in range(2):
                            kb = 4 * j + 2 * half + rr
                            nc.tensor.matmul(
                                psh[:, rr, :],
                                lhsT=kvT[0:64, kb * 128:(kb + 1) * 128],
                                rhs=q_ap, start=True, stop=True)
                        nc.scalar.activation(
                            out=exT[:, 2 * half:2 * half + 2, :], in_=psh,
                            func=mybir.ActivationFunctionType.Exp, scale=SCALE)
                    if j == a:  # diagonal of slot 2p
                        nc.vector.tensor_mul(
                            exT[:, 0:2, 0:HGS], exT[:, 0:2, 0:HGS], tri_sb)
                        nc.vector.tensor_scalar_mul(
                            exT[:, 2:4, 0:HGS], exT[:, 2:4, 0:HGS],
                            dsel_sb[:, a:a + 1])
                    pending.append((exT, j, 2, uvp, av_start, av_stop, None))

                def sitem_front(b, uvp, av_start, av_stop):
                    """Solo diagonal item for odd slot b at key-group j=b."""
                    j = b
                    q_ap = qT[:, b * HGS:(b + 1) * HGS]
                    exT = expool.tile([128, 4, HGS], dt.bfloat16, tag="ex",
                                      name="exs")
                    ps4 = pscore.tile([128, 4, HGS], dt.float32, tag="sc",
                                      name="ps_s")
                    for r in range(4):
                        kb = 4 * j + r
                        nc.tensor.matmul(
                            ps4[:, r, :],
                            lhsT=kvT[0:64, kb * 128:(kb + 1) * 128],
                            rhs=q_ap, start=True, stop=True)
                    nc.scalar.activation(
                        out=exT, in_=ps4,
                        func=mybir.ActivationFunctionType.Exp, scale=SCALE)
                    nc.vector.tensor_mul(exT[:, 0:2, :], exT[:, 0:2, :], tri_sb)
                    nc.vector.tensor_scalar_mul(
                        exT[:, 2:4, :], exT[:, 2:4, :], dsel_sb[:, b:b + 1])
                    pending.append((exT, j, 1, uvp, av_start, av_stop, None))

                def epilogue_pair(p, uvp):
                    """Both slots of pair p: PSUM->SBUF, transpose, scale, DMA."""
                    u_sb = epi.tile([H + 1, 2 * HGS], dt.float32, tag="usb")
                    nc.vector.tensor_copy(out=u_sb, in_=uvp)
                    psts = []
                    for hh in range(4):
                        pst = paux.tile([128, H + 1], dt.float32, tag="a",
                                        name="pst_ep")
                        nc.tensor.transpose(
                            pst, u_sb[:, hh * 128:(hh + 1) * 128],
                            identity32[:, :])
                        psts.append(pst)
                    for hh, pst in enumerate(psts):
                        s = 2 * p + hh // 2
                        rec = epi.tile([128, 1], dt.float32, tag="rec")
                        nc.vector.reciprocal(rec, pst[:, H:H + 1])
                        o_sb = epi.tile([128, H], dt.float32, tag="o")
                        nc.vector.tensor_scalar_mul(o_sb, pst[:, 0:H], rec)
                        row0 = s * HGS + (hh % 2) * 128
                        nc.sync.dma_start(out=out_e[row0:row0 + 128, :],
                                          in_=o_sb)

                # ---- emission schedule ----
                # Full input-DMA ring up front on SP, in feed order, so no
                # out-DMA wait ever blocks an input DMA on SP.SEQ.
                dma_w()
                qdma(0, 0)
                qdma(0, 1)
                dma_w2()
                fdma(0)
                qdma(1, 0)
                fdma(1)
                qdma(1, 1)
                qdma(2, 0)
                fdma(2)
                qdma(2, 1)
                qdma(3, 0)
                fdma(3)
                qdma(3, 1)
                for j in range(4, NSLOT):
                    fdma(j)

                # PE warmup on identity while the first DMAs stream in.
                for i in range(N_WARM):
                    pw = paux.tile([128, 128], dt.float32, tag="a", name="warm")
                    nc.tensor.matmul(pw, lhsT=identity, rhs=identity,
                                     start=True, stop=True)

                # pair item streams: pair p covers items j=0..2p paired plus
                # the solo diagonal of slot 2p+1; AV accumulates in uvp.
                uv_tiles = {}

                def start_pair(p):
                    uv_tiles[p] = pu.tile([H + 1, 2 * HGS], dt.float32, tag="u",
                                          name=f"uv{p}")

                def pair_item(p, j):
                    pitem_front(p, j, uv_tiles[p], j == 0, False)
                    while len(pending) > 1:
                        flush_av()

                def pair_solo(p):
                    sitem_front(2 * p + 1, uv_tiles[p], False, True)
                    while len(pending) > 1:
                        flush_av()

                own(0)
                v1own(0)
                own(1)
                v1own(1)
                foreign(0)
                start_pair(0)
                pair_item(0, 0)
                own(2)
                v1own(2)
                foreign(1)
                pair_solo(0)
                own(3)
                v1own(3)
                while pending:
                    flush_av()
                epilogue_pair(0, uv_tiles.pop(0))

                start_pair(1)
                pair_item(1, 0)
                own(4)
                v1own(4)
                pair_item(1, 1)
                foreign(2)
                own(5)
                v1own(5)
                pair_item(1, 2)
                foreign(3)
                own(6)
                v1own(6)
                pair_solo(1)
                own(7)
                v1own(7)
                while pending:
                    flush_av()
                epilogue_pair(1, uv_tiles.pop(1))

                start_pair(2)
                pair_item(2, 0)
                foreign(4)
                pair_item(2, 1)
                foreign(5)
                pair_item(2, 2)
                foreign(6)
                pair_item(2, 3)
                foreign(7)
                pair_item(2, 4)
                pair_solo(2)
                while pending:
                    flush_av()
                epilogue_pair(2, uv_tiles.pop(2))

                start_pair(3)
                for j in range(5):
                    pair_item(3, j)
                sitem_front(7, uv_tiles[3], False, False)  # solo mid-stream
                while len(pending) > 1:
                    flush_av()
                pair_item(3, 5)
                pitem_front(3, 6, uv_tiles[3], False, True)  # stop on last
                while pending:
                    flush_av()
                epilogue_pair(3, uv_tiles.pop(3))
    nc.compile()
    return nc


def _host_inputs(Wk, Wq, Wv):
    wkv = _bf16(np.concatenate([Wk, Wv], axis=1))
    wkq = _bf16(np.concatenate([Wk, Wq], axis=1))
    rk = np.arange(HGS)[:, None]
    cq = np.arange(HGS)[None, :]
    tri = (rk <= cq).astype(np.float32)           # [256, 256] own triangle
    tri = _bf16(tri.reshape(2, 128, HGS).transpose(1, 0, 2).reshape(128, 2 * HGS))
    dsel = {}
    for half, hgs in ((0, HGS_A), (1, HGS_B)):
        d = np.array([[1.0 if hg % 2 == 1 else 0.0 for hg in hgs]] * 128,
                     dtype=np.float32)
        dsel[half] = np.ascontiguousarray(d)
    return wkv, wkq, tri, dsel


def kernel(x, Wk, Wq, Wv):
    from concourse.bass_utils import run_bass_kernel_spmd

    x = np.asarray(x, dtype=np.float32)
    Wk = np.asarray(Wk, dtype=np.float32)
    Wq = np.asarray(Wq, dtype=np.float32)
    Wv = np.asarray(Wv, dtype=np.float32)

    if "nc" not in _cache:
        _cache["nc"] = _build_graph()
    nc = _cache["nc"]

    wkv, wkq, tri, dsel = _host_inputs(Wk, Wq, Wv)

    in_maps = []
    core_meta = []
    for b in range(B):
        xTb = _bf16(x[b].T)  # [E, T]
        for half, hgs in enumerate([HGS_A, HGS_B]):
            other = [HGS_A, HGS_B][1 - half]
            xp = np.concatenate(
                [xTb[:, hg * HGS:(hg + 1) * HGS] for hg in list(hgs) + other],
                axis=1)
            in_maps.append({
                "xT": np.ascontiguousarray(xp),
                "wkv": wkv,
                "wkq": wkq,
                "tri": tri,
                "dsel": dsel[half],
            })
            core_meta.append((b, hgs))

    res = run_bass_kernel_spmd(nc, in_maps, core_ids=list(range(8)),
                               **_cache.get("run_kwargs", {}))
    _cache["last_result"] = res

    full = np.zeros((B, T, H), dtype=np.float32)
    for core, (b, hgs) in enumerate(core_meta):
        o = res.results[core]["out"]
        for s, hg in enumerate(hgs):
            full[b, hg * HGS:(hg + 1) * HGS, :] = o[s * HGS:(s + 1) * HGS, :]
    return full


# revision 16
# speedup vs baseline: 1.1469x; 1.0415x over previous
"""Causal single-head attention (B=4, T=4096, E=1024, H=64) on 8 TRN2 cores.

Sharding: 2 cores per batch; no collectives (host shards, device computes,
host gathers). Queries are assigned to cores in 256-row half-groups with the
fold pattern {0,3}/{1,2} (mod 4), which makes both cores' causal work-lists
IDENTICAL: 8 query slots with key-group trip counts exactly (1..8), so one
SPMD graph serves all cores; all per-core variation (which queries, causal
mask content, key order) lives in host-prepared input data.

Host prep (layout-only, no FLOPs): x[b]^T cast to bf16 with columns permuted
to [owned half-groups in slot order | partner half-groups in the other
core's slot order]. Slot j's 256 queries are exactly the own half of
key-group j, so one fused [Wk|Wq] projection pass over the own columns
yields both K^T and Q^T; V1 for own tokens is computed directly
(lhsT=x-block, rhs=Wv) with full 128-partition output. Foreign columns get
a [Wk|Wv] pass + PE transposes for V1. The diagonal causal mask is a single
shared 256x256 triangle (identical for every slot and core) plus a per-slot
0/1 parity scalar for the foreign half -- tiny inputs instead of per-slot
masks.

Device (bf16 compute, f32 PSUM): items (pair p, key-group j) stream
pair-major; each pair's attn@V accumulates IN PSUM across its whole item
stream (matmul start/stop flags span items), so there are no per-item
accumulate ops at all. exp on ACT with scale=E^-0.5 folded in; V1 carries a
ones column so the softmax denominator falls out of the attn@V matmul.
Epilogue per pair: PSUM->SBUF copy, PE-transpose, reciprocal * scale, DMA.
Input DMAs ride SP in feed order; PE warms up on identity matmuls while the
first DMAs land.
"""
import numpy as np
import ml_dtypes

B, T, E, H = 4, 4096, 1024, 64
HGS = 256         # queries per slot (half-group size)
KG = 512          # keys per key-group
NSLOT = 8
NQ = NSLOT * HGS  # 2048 owned queries per core
ET = E // 128     # 8 E-tiles
NKB = T // 128    # 32 key blocks
SCALE = float(E) ** -0.5
N_WARM = 26       # PE warmup matmuls (N=128) while first DMAs land

HGS_A = [0, 3, 4, 7, 8, 11, 12, 15]   # core half 0: needs 1..8 in slot order
HGS_B = [1, 2, 5, 6, 9, 10, 13, 14]   # core half 1: needs 1..8 in slot order

_cache = {}


def _bf16(a):
    return np.ascontiguousarray(a.astype(ml_dtypes.bfloat16))


def _build_graph():
    import concourse.mybir as mybir
    import concourse.tile as tile
    from concourse import bacc
    from concourse.masks import make_identity

    dt = mybir.dt
    nc = bacc.Bacc(None, target_bir_lowering=False)
    xT_e = nc.declare_dram_parameter("xT", [E, T], dt.bfloat16, isOutput=False)
    wkv_e = nc.declare_dram_parameter("wkv", [E, 128], dt.bfloat16, isOutput=False)
    wkq_e = nc.declare_dram_parameter("wkq", [E, 128], dt.bfloat16, isOutput=False)
    tri_e = nc.declare_dram_parameter("tri", [128, 2 * HGS], dt.bfloat16,
                                      isOutput=False)
    dsel_e = nc.declare_dram_parameter("dsel", [128, NSLOT], dt.float32,
                                       isOutput=False)
    out_e = nc.declare_dram_parameter("out", [NQ, H], dt.float32, isOutput=True)

    xT_r = xT_e.rearrange("(et p) t -> p et t", p=128)

    with tile.TileContext(nc) as tc:
        with (
            tc.tile_pool(name="singles", bufs=1) as singles,
            tc.tile_pool(name="persist", bufs=1) as persist,
        ):
            identity = singles.tile([128, 128], dt.bfloat16)
            make_identity(nc, identity)
            identity32 = singles.tile([H + 1, H + 1], dt.float32)
            make_identity(nc, identity32)
            wkv_sb = singles.tile([128, ET, 128], dt.bfloat16)
            wkq_sb = singles.tile([128, ET, 128], dt.bfloat16)
            tri_sb = singles.tile([128, 2, HGS], dt.bfloat16)
            dsel_sb = singles.tile([128, NSLOT], dt.float32)

            # persistent activations
            kvT = persist.tile([128, T], dt.bfloat16)    # 0:64 K^T, 64:128 V^T(frn)
            qT = persist.tile([64, NQ], dt.bfloat16)
            v1 = persist.tile([128, NKB, H + 1], dt.bfloat16)
            # per pair g: columns [own_2g | foreign_2g | own_2g+1 | foreign_2g+1]
            xq_tiles = [persist.tile([128, ET, 4, HGS], dt.bfloat16,
                                     name=f"xq{g}") for g in range(4)]

            nc.vector.memset(v1[:, :, H], 1.0)  # denominator ones column

            with (
                tc.tile_pool(name="pscore", bufs=2, space="PSUM") as pscore,
                tc.tile_pool(name="paux", bufs=2, space="PSUM") as paux,
                tc.tile_pool(name="pu", bufs=2, space="PSUM") as pu,
                tc.tile_pool(name="ex", bufs=3) as expool,
                tc.tile_pool(name="epi", bufs=4) as epi,
            ):
                # ---- DMA issue helpers (all inputs on SP, feed order) ----
                def dma_wkq():
                    nc.sync.dma_start(
                        out=wkq_sb, in_=wkq_e.rearrange("(et p) m -> p et m", p=128))

                def dma_wkv():
                    nc.sync.dma_start(
                        out=wkv_sb, in_=wkv_e.rearrange("(et p) m -> p et m", p=128))

                def dma_tri():
                    nc.sync.dma_start(out=tri_sb,
                                      in_=tri_e.rearrange("p (r c) -> p r c", r=2))
                    nc.sync.dma_start(out=dsel_sb, in_=dsel_e[:, :])

                def qdma(g, two, split=1):
                    # own half for key-group 2g+two -> c-slot 2*two
                    step = ET // split
                    for h in range(split):
                        nc.sync.dma_start(
                            out=xq_tiles[g][:, h * step:(h + 1) * step, 2 * two, :],
                            in_=xT_r[:, h * step:(h + 1) * step,
                                     g * KG + two * HGS:g * KG + (two + 1) * HGS])

                def fdma(j):
                    # foreign half for key-group j -> c-slot 2*(j%2)+1
                    nc.sync.dma_start(
                        out=xq_tiles[j // 2][:, :, 2 * (j % 2) + 1, :],
                        in_=xT_r[:, :, NQ + j * HGS:NQ + (j + 1) * HGS])

                # ---- projection passes ----
                def own(j):
                    """[Wk|Wq] over own cols of key-group j: K^T own half +
                    Q^T of slot j (slot j's queries ARE its own keys)."""
                    xo = xq_tiles[j // 2][:, :, 2 * (j % 2), :]
                    ps = paux.tile([128, HGS], dt.float32, tag="a")
                    for et in range(ET):
                        nc.tensor.matmul(ps, lhsT=wkq_sb[:, et, :],
                                         rhs=xo[:, et, :],
                                         start=(et == 0), stop=(et == ET - 1))
                    nc.vector.tensor_copy(out=kvT[0:64, j * KG:j * KG + HGS],
                                          in_=ps[0:64, :])
                    nc.vector.tensor_copy(out=qT[:, j * HGS:(j + 1) * HGS],
                                          in_=ps[64:128, :])

                def v1own(j):
                    """V1 for own tokens of key-group j, directly:
                    out[tok,H] = sum_et x_blk^T.T @ Wv_et (full-M, free=64)."""
                    xo = xq_tiles[j // 2][:, :, 2 * (j % 2), :]
                    for b in range(2):
                        psv = paux.tile([128, H], dt.float32, tag="a", name="psv")
                        for et in range(ET):
                            nc.tensor.matmul(
                                psv, lhsT=xo[:, et, b * 128:(b + 1) * 128],
                                rhs=wkv_sb[:, et, 64:128],
                                start=(et == 0), stop=(et == ET - 1))
                        nc.vector.tensor_copy(out=v1[:, 4 * j + b, 0:H], in_=psv)

                def foreign(j):
                    """[Wk|Wv] over foreign cols of key-group j, then PE
                    transposes of V^T into V1 blocks 4j+2, 4j+3."""
                    xf = xq_tiles[j // 2][:, :, 2 * (j % 2) + 1, :]
                    ps = paux.tile([128, HGS], dt.float32, tag="a")
                    for et in range(ET):
                        nc.tensor.matmul(ps, lhsT=wkv_sb[:, et, :],
                                         rhs=xf[:, et, :],
                                         start=(et == 0), stop=(et == ET - 1))
                    nc.vector.tensor_copy(
                        out=kvT[:, j * KG + HGS:(j + 1) * KG], in_=ps)
                    for b in range(2):
                        kb = 4 * j + 2 + b
                        pst = paux.tile([128, H], dt.bfloat16, tag="a",
                                       name="pst_vt")
                        nc.tensor.transpose(
                            pst, kvT[64:128, kb * 128:(kb + 1) * 128],
                            identity[64:128, 64:128])
                        nc.vector.tensor_copy(out=v1[:, kb, 0:H], in_=pst)

                # ---- attention items, pair-major with PSUM-resident acc ----
                # pending: (exT, j, width, uv, av_start, av_stop, diag_slot)
                pending = []

                def flush_av():
                    exT, j, w, uv, av_start, av_stop, _ = pending.pop(0)
                    # solo items (w=1) cover only the odd slot's column half
                    uvs = uv if w == 2 else uv[:, HGS:2 * HGS]
                    for r in range(4):
                        nc.tensor.matmul(
                            uvs, lhsT=v1[:, 4 * j + r, :], rhs=exT[:, r, :],
                            start=(av_start and r == 0),
                            stop=(av_stop and r == 3),
                            skip_group_check=True)

                def pitem_front(p, j, uvp, av_start, av_stop):
                    """Paired item: slots (2p, 2p+1), key-group j, N=512.
                    Diagonal-masked on slot 2p's half when j == 2p."""
                    a = 2 * p
                    q_ap = qT[:, a * HGS:(a + 2) * HGS]
                    exT = expool.tile([128, 4, 2 * HGS], dt.bfloat16, tag="ex")
                    for half in range(2):
                        psh = pscore.tile([128, 2, 2 * HGS], dt.float32,
                                          tag="sc", name="ps_h")
                        for rr in range(2):
                            kb = 4 * j + 2 * half + rr
                            nc.tensor.matmul(
                                psh[:, rr, :],
                                lhsT=kvT[0:64, kb * 128:(kb + 1) * 128],
                                rhs=q_ap, start=True, stop=True)
                        nc.scalar.activation(
                            out=exT[:, 2 * half:2 * half + 2, :], in_=psh,
                            func=mybir.ActivationFunctionType.Exp, scale=SCALE)
                    if j == a:  # diagonal of slot 2p
                        nc.vector.tensor_mul(
                            exT[:, 0:2, 0:HGS], exT[:, 0:2, 0:HGS], tri_sb)
                        nc.vector.tensor_scalar_mul(
                            exT[:, 2:4, 0:HGS], exT[:, 2:4, 0:HGS],
                            dsel_sb[:, a:a + 1])
                    pending.append((exT, j, 2, uvp, av_start, av_stop, None))

                def sitem_front(b, uvp, av_start, av_stop):
                    """Solo diagonal item for odd slot b at key-group j=b."""
                    j = b
                    q_ap = qT[:, b * HGS:(b + 1) * HGS]
                    exT = expool.tile([128, 4, HGS], dt.bfloat16, tag="ex",
                                      name="exs")
                    ps4 = pscore.tile([128, 4, HGS], dt.float32, tag="sc",
                                      name="ps_s")
                    for r in range(4):
                        kb = 4 * j + r
                        nc.tensor.matmul(
                            ps4[:, r, :],
                            lhsT=kvT[0:64, kb * 128:(kb + 1) * 128],
                            rhs=q_ap, start=True, stop=True)
                    nc.scalar.activation(
                        out=exT, in_=ps4,
                        func=mybir.ActivationFunctionType.Exp, scale=SCALE)
                    nc.vector.tensor_mul(exT[:, 0:2, :], exT[:, 0:2, :], tri_sb)
                    nc.vector.tensor_scalar_mul(
                        exT[:, 2:4, :], exT[:, 2:4, :], dsel_sb[:, b:b + 1])
                    pending.append((exT, j, 1, uvp, av_start, av_stop, None))

                def epilogue_slot(s, uvp, col0):
                    """One slot: PSUM->SBUF, 2x transpose, scale, DMA.
                    Output DMAs alternate SP/ACT so the last pair's four
                    issues don't serialize on one sequencer."""
                    u_sb = epi.tile([H + 1, HGS], dt.float32, tag="usb")
                    nc.vector.tensor_copy(out=u_sb,
                                          in_=uvp[:, col0:col0 + HGS])
                    psts = []
                    for hh in range(2):
                        pst = paux.tile([128, H + 1], dt.float32, tag="a",
                                        name="pst_ep")
                        nc.tensor.transpose(
                            pst, u_sb[:, hh * 128:(hh + 1) * 128],
                            identity32[:, :])
                        psts.append(pst)
                    o_sb = epi.tile([128, 2, H], dt.float32, tag="o")
                    for hh, pst in enumerate(psts):
                        rec = epi.tile([128, 1], dt.float32, tag="rec")
                        nc.vector.reciprocal(rec, pst[:, H:H + 1])
                        nc.vector.tensor_scalar_mul(o_sb[:, hh, :], pst[:, 0:H],
                                                    rec)
                    nc.sync.dma_start(
                        out=out_e[s * HGS:(s + 1) * HGS, :]
                        .rearrange("(hh p) h -> p hh h", hh=2),
                        in_=o_sb)

                # ---- emission schedule ----
                # Input-DMA ring on SP in feed order (transfer-serialized on
                # the DMA engines, so issue order = arrival order).
                dma_wkq()
                qdma(0, 0, split=2)
                dma_wkv()
                qdma(0, 1)
                qdma(1, 0)
                dma_tri()
                fdma(0)
                fdma(1)
                qdma(1, 1)
                qdma(2, 0)
                fdma(2)
                qdma(2, 1)
                fdma(3)
                qdma(3, 0)
                qdma(3, 1)
                for j in range(4, NSLOT):
                    fdma(j)

                # PE warmup on identity while the first DMAs stream in.
                for i in range(N_WARM):
                    pw = paux.tile([128, 128], dt.float32, tag="a", name="warm")
                    nc.tensor.matmul(pw, lhsT=identity, rhs=identity,
                                     start=True, stop=True)

                # pair item streams: pair p = paired j=0..2p + solo diagonal
                # of slot 2p+1; AV accumulates in one PSUM tile per pair.
                uv_tiles = {}

                def start_pair(p):
                    uv_tiles[p] = pu.tile([H + 1, 2 * HGS], dt.float32, tag="u",
                                          name=f"uv{p}")

                def front(p, j):
                    if p not in uv_tiles:
                        start_pair(p)
                    pitem_front(p, j, uv_tiles[p], j == 0, j == 2 * p)
                    while len(pending) > 1:
                        flush_av()

                def front_solo(p):
                    sitem_front(2 * p + 1, uv_tiles[p], False, True)
                    while len(pending) > 1:
                        flush_av()

                def drain():
                    while pending:
                        flush_av()

                # emission order follows predicted data-arrival order (PE
                # executes its queue in order, so this IS the PE schedule)
                own(0)
                v1own(0)
                own(1)
                v1own(1)
                own(2)
                v1own(2)
                foreign(0)
                front(0, 0)
                foreign(1)
                own(3)
                v1own(3)
                front_solo(0)          # flushes (0,0) AV
                epilogue_slot(0, uv_tiles[0], 0)
                own(4)
                v1own(4)
                front(1, 0)            # flushes solo(1) AV
                epilogue_slot(1, uv_tiles.pop(0), HGS)
                front(1, 1)
                foreign(2)
                own(5)
                v1own(5)
                front(1, 2)
                foreign(3)
                own(6)
                v1own(6)
                front_solo(1)          # flushes (1,2) AV
                epilogue_slot(2, uv_tiles[1], 0)
                own(7)
                v1own(7)
                front(2, 0)            # flushes solo(3) AV
                epilogue_slot(3, uv_tiles.pop(1), HGS)
                foreign(4)
                front(2, 1)
                foreign(5)
                front(2, 2)
                foreign(6)
                front(2, 3)
                foreign(7)
                front(2, 4)
                front_solo(2)          # flushes (2,4) AV
                epilogue_slot(4, uv_tiles[2], 0)
                front(3, 0)            # flushes solo(5) AV
                epilogue_slot(5, uv_tiles.pop(2), HGS)
                for j in range(1, 7):
                    front(3, j)
                front_solo(3)          # flushes (3,6) AV
                epilogue_slot(6, uv_tiles[3], 0)
                drain()                # solo(7) AV
                epilogue_slot(7, uv_tiles.pop(3), HGS)
    nc.compile()
    return nc


def _host_inputs(Wk, Wq, Wv):
    wkv = _bf16(np.concatenate([Wk, Wv], axis=1))
    wkq = _bf16(np.concatenate([Wk, Wq], axis=1))
    rk = np.arange(HGS)[:, None]
    cq = np.arange(HGS)[None, :]
    tri = (rk <= cq).astype(np.float32)           # [256, 256] own triangle
    tri = _bf16(tri.reshape(2, 128, HGS).transpose(1, 0, 2).reshape(128, 2 * HGS))
    dsel = {}
    for half, hgs in ((0, HGS_A), (1, HGS_B)):
        d = np.array([[1.0 if hg % 2 == 1 else 0.0 for hg in hgs]] * 128,
                     dtype=np.float32)
        dsel[half] = np.ascontiguousarray(d)
    return wkv, wkq, tri, dsel


def kernel(x, Wk, Wq, Wv):
    from concourse.bass_utils import run_bass_kernel_spmd

    x = np.asarray(x, dtype=np.float32)
    Wk = np.asarray(Wk, dtype=np.float32)
    Wq = np.asarray(Wq, dtype=np.float32)
    Wv = np.asarray(Wv, dtype=np.float32)

    if "nc" not in _cache:
        _cache["nc"] = _build_graph()
    nc = _cache["nc"]

    wkv, wkq, tri, dsel = _host_inputs(Wk, Wq, Wv)

    in_maps = []
    core_meta = []
    for b in range(B):
        xTb = _bf16(x[b].T)  # [E, T]
        for half, hgs in enumerate([HGS_A, HGS_B]):
            other = [HGS_A, HGS_B][1 - half]
            xp = np.concatenate(
                [xTb[:, hg * HGS:(hg + 1) * HGS] for hg in list(hgs) + other],
                axis=1)
            in_maps.append({
                "xT": np.ascontiguousarray(xp),
                "wkv": wkv,
                "wkq": wkq,
                "tri": tri,
                "dsel": dsel[half],
            })
            core_meta.append((b, hgs))

    res = run_bass_kernel_spmd(nc, in_maps, core_ids=list(range(8)),
                               **_cache.get("run_kwargs", {}))
    _cache["last_result"] = res

    full = np.zeros((B, T, H), dtype=np.float32)
    for core, (b, hgs) in enumerate(core_meta):
        o = res.results[core]["out"]
        for s, hg in enumerate(hgs):
            full[b, hg * HGS:(hg + 1) * HGS, :] = o[s * HGS:(s + 1) * HGS, :]
    return full


# revision 35
# speedup vs baseline: 1.2276x; 1.0704x over previous
"""Causal single-head attention (B=4, T=4096, E=1024, H=64) on 8 TRN2 cores.

Sharding: 2 cores per batch; no collectives (host shards, device computes,
host gathers). Queries are assigned to cores in 256-row half-groups with the
fold pattern {0,3}/{1,2} (mod 4), which makes both cores' causal work-lists
IDENTICAL: 8 query slots with key-group trip counts exactly (1..8), so one
SPMD graph serves all cores; all per-core variation (which queries, causal
mask content, key order) lives in host-prepared input data.

Host prep (layout-only, no FLOPs): x[b]^T cast to bf16 with columns permuted
to [owned half-groups in slot order | partner half-groups in the other
core's slot order]. Slot j's 256 queries are exactly the own half of
key-group j, so one fused [Wk|Wq] projection pass over the own columns
yields both K^T and Q^T; V1 for own tokens is computed directly
(lhsT=x-block, rhs=Wv) with full 128-partition output. Foreign columns get
a [Wk|Wv] pass + PE transposes for V1. The diagonal causal mask is a single
shared 256x256 triangle (identical for every slot and core) plus a per-slot
0/1 parity scalar for the foreign half -- tiny inputs instead of per-slot
masks.

Device (bf16 compute, f32 PSUM): items (pair p, key-group j) stream
pair-major; each pair's attn@V accumulates IN PSUM across its whole item
stream (matmul start/stop flags span items), so there are no per-item
accumulate ops at all. exp on ACT with scale=E^-0.5 folded in; V1 carries a
ones column so the softmax denominator falls out of the attn@V matmul.
Epilogue per slot: PSUM->SBUF copy, PE-transpose, reciprocal * scale, one
partition-major DMA. Input DMAs ride SP in arrival==consumption order
(weights pre-tiled host-side to dodge the sub-512B DMA penalty); PE warms up
on a memset tile while the first DMAs land. Emission order is hand-scheduled
against the DMA arrival timeline since each engine executes its queue
in order.
"""
import numpy as np
import ml_dtypes

B, T, E, H = 4, 4096, 1024, 64
HGS = 256         # queries per slot (half-group size)
KG = 512          # keys per key-group
NSLOT = 8
NQ = NSLOT * HGS  # 2048 owned queries per core
ET = E // 128     # 8 E-tiles
NKB = T // 128    # 32 key blocks
SCALE = float(E) ** -0.5
N_WARM = 26       # PE warmup matmuls (N=128) while first DMAs land

HGS_A = [0, 3, 4, 7, 8, 11, 12, 15]   # core half 0: needs 1..8 in slot order
HGS_B = [1, 2, 5, 6, 9, 10, 13, 14]   # core half 1: needs 1..8 in slot order

_cache = {}


def _bf16(a):
    return np.ascontiguousarray(a.astype(ml_dtypes.bfloat16))


def _build_graph():
    import concourse.mybir as mybir
    import concourse.tile as tile
    from concourse import bacc
    from concourse.masks import make_identity

    dt = mybir.dt
    nc = bacc.Bacc(None, target_bir_lowering=False)
    xT_e = nc.declare_dram_parameter("xT", [E, T], dt.bfloat16, isOutput=False)
    wkv_e = nc.declare_dram_parameter("wkv", [128, ET * 128], dt.bfloat16,
                                      isOutput=False)
    wkq_e = nc.declare_dram_parameter("wkq", [128, ET * 128], dt.bfloat16,
                                      isOutput=False)
    tri_e = nc.declare_dram_parameter("tri", [128, 2 * HGS], dt.bfloat16,
                                      isOutput=False)
    dsel_e = nc.declare_dram_parameter("dsel", [128, NSLOT], dt.float32,
                                       isOutput=False)
    out_e = nc.declare_dram_parameter("out", [128, NSLOT * 2 * H], dt.float32,
                                      isOutput=True)

    xT_r = xT_e.rearrange("(et p) t -> p et t", p=128)

    with tile.TileContext(nc) as tc:
        with (
            tc.tile_pool(name="singles", bufs=1) as singles,
            tc.tile_pool(name="persist", bufs=1) as persist,
        ):
            identity = singles.tile([128, 128], dt.bfloat16)
            make_identity(nc, identity)
            identity32 = singles.tile([H + 1, H + 1], dt.float32)
            make_identity(nc, identity32)
            wkv_sb = singles.tile([128, ET, 128], dt.bfloat16)
            wkq_sb = singles.tile([128, ET, 128], dt.bfloat16)
            tri_sb = singles.tile([128, 2, HGS], dt.bfloat16)
            dsel_sb = singles.tile([128, NSLOT], dt.float32)

            # persistent activations
            kvT = persist.tile([128, T], dt.bfloat16)    # 0:64 K^T, 64:128 V^T(frn)
            qT = persist.tile([64, NQ], dt.bfloat16)
            v1 = persist.tile([128, NKB, H + 1], dt.bfloat16)
            # per pair g: columns [own_2g | foreign_2g | own_2g+1 | foreign_2g+1]
            xq_tiles = [persist.tile([128, ET, 4, HGS], dt.bfloat16,
                                     name=f"xq{g}") for g in range(4)]

            nc.vector.memset(v1[:, :, H], 1.0)  # denominator ones column

            with (
                tc.tile_pool(name="pscore", bufs=2, space="PSUM") as pscore,
                tc.tile_pool(name="paux", bufs=2, space="PSUM") as paux,
                tc.tile_pool(name="pu", bufs=2, space="PSUM") as pu,
                tc.tile_pool(name="ex", bufs=4) as expool,
                tc.tile_pool(name="epi", bufs=4) as epi,
            ):
                # ---- DMA issue helpers (all inputs on SP, feed order) ----
                def dma_wkq():
                    nc.sync.dma_start(
                        out=wkq_sb,
                        in_=wkq_e.rearrange("p (et m) -> p et m", et=ET))

                def dma_wkv():
                    nc.sync.dma_start(
                        out=wkv_sb, in_=wkv_e.rearrange("p (et m) -> p et m", et=ET))

                def dma_tri():
                    nc.sync.dma_start(out=tri_sb,
                                      in_=tri_e.rearrange("p (r c) -> p r c", r=2))
                    nc.sync.dma_start(out=dsel_sb, in_=dsel_e[:, :])

                def qdma(g, two, split=1):
                    # own half for key-group 2g+two -> c-slot 2*two
                    step = ET // split
                    for h in range(split):
                        nc.sync.dma_start(
                            out=xq_tiles[g][:, h * step:(h + 1) * step, 2 * two, :],
                            in_=xT_r[:, h * step:(h + 1) * step,
                                     g * KG + two * HGS:g * KG + (two + 1) * HGS])

                def fdma(j):
                    # foreign half for key-group j -> c-slot 2*(j%2)+1
                    nc.sync.dma_start(
                        out=xq_tiles[j // 2][:, :, 2 * (j % 2) + 1, :],
                        in_=xT_r[:, :, NQ + j * HGS:NQ + (j + 1) * HGS])

                # ---- projection passes ----
                def own(j):
                    """[Wk|Wq] over own cols of key-group j: K^T own half +
                    Q^T of slot j (slot j's queries ARE its own keys)."""
                    xo = xq_tiles[j // 2][:, :, 2 * (j % 2), :]
                    ps = paux.tile([128, HGS], dt.float32, tag="a")
                    for et in range(ET):
                        nc.tensor.matmul(ps, lhsT=wkq_sb[:, et, :],
                                         rhs=xo[:, et, :],
                                         start=(et == 0), stop=(et == ET - 1))
                    nc.vector.tensor_copy(out=kvT[0:64, j * KG:j * KG + HGS],
                                          in_=ps[0:64, :])
                    nc.vector.tensor_copy(out=qT[:, j * HGS:(j + 1) * HGS],
                                          in_=ps[64:128, :])

                def v1own(j):
                    """V1 for own tokens of key-group j, directly:
                    out[tok,H] = sum_et x_blk^T.T @ Wv_et (full-M, free=64)."""
                    xo = xq_tiles[j // 2][:, :, 2 * (j % 2), :]
                    for b in range(2):
                        psv = paux.tile([128, H], dt.float32, tag="a", name="psv")
                        for et in range(ET):
                            nc.tensor.matmul(
                                psv, lhsT=xo[:, et, b * 128:(b + 1) * 128],
                                rhs=wkv_sb[:, et, 64:128],
                                start=(et == 0), stop=(et == ET - 1))
                        nc.vector.tensor_copy(out=v1[:, 4 * j + b, 0:H], in_=psv)

                def foreign(j):
                    """[Wk|Wv] over foreign cols of key-group j, then PE
                    transposes of V^T into V1 blocks 4j+2, 4j+3."""
                    xf = xq_tiles[j // 2][:, :, 2 * (j % 2) + 1, :]
                    ps = paux.tile([128, HGS], dt.float32, tag="a")
                    for et in range(ET):
                        nc.tensor.matmul(ps, lhsT=wkv_sb[:, et, :],
                                         rhs=xf[:, et, :],
                                         start=(et == 0), stop=(et == ET - 1))
                    nc.vector.tensor_copy(
                        out=kvT[:, j * KG + HGS:(j + 1) * KG], in_=ps)
                    for b in range(2):
                        kb = 4 * j + 2 + b
                        pst = paux.tile([128, H], dt.bfloat16, tag="a",
                                       name="pst_vt")
                        nc.tensor.transpose(
                            pst, kvT[64:128, kb * 128:(kb + 1) * 128],
                            identity[64:128, 64:128])
                        nc.vector.tensor_copy(out=v1[:, kb, 0:H], in_=pst)

                # ---- attention items, pair-major with PSUM-resident acc ----
                # pending: (exT, j, width, uv, av_start, av_stop, epi)
                pending = []

                def flush_av():
                    exT, j, w, uv, av_start, av_stop, epi_m = pending.pop(0)
                    uvs = uv if w == 2 else uv[:, HGS:2 * HGS]
                    for r in range(4):
                        nc.tensor.matmul(
                            uvs, lhsT=v1[:, 4 * j + r, :], rhs=exT[:, r, :],
                            start=(av_start and r == 0),
                            stop=(av_stop and r == 3),
                            skip_group_check=True)
                    if epi_m is not None:
                        epilogue_slot(*epi_m)

                def pitem_front(p, j, uvp, av_start, av_stop, epi_m):
                    """Paired item: slots (2p, 2p+1), key-group j, N=512.
                    Diagonal-masked on slot 2p's half when j == 2p."""
                    a = 2 * p
                    q_ap = qT[:, a * HGS:(a + 2) * HGS]
                    exT = expool.tile([128, 4, 2 * HGS], dt.bfloat16, tag="ex")
                    for half in range(2):
                        psh = pscore.tile([128, 2, 2 * HGS], dt.float32,
                                          tag="sc", name="ps_h")
                        for rr in range(2):
                            kb = 4 * j + 2 * half + rr
                            nc.tensor.matmul(
                                psh[:, rr, :],
                                lhsT=kvT[0:64, kb * 128:(kb + 1) * 128],
                                rhs=q_ap, start=True, stop=True)
                        nc.scalar.activation(
                            out=exT[:, 2 * half:2 * half + 2, :], in_=psh,
                            func=mybir.ActivationFunctionType.Exp, scale=SCALE)
                        if j == a and half == 0:
                            nc.vector.tensor_mul(
                                exT[:, 0:2, 0:HGS], exT[:, 0:2, 0:HGS], tri_sb)
                    if j == a:  # diagonal foreign half of slot 2p
                        nc.vector.tensor_scalar_mul(
                            exT[:, 2:4, 0:HGS], exT[:, 2:4, 0:HGS],
                            dsel_sb[:, a:a + 1])
                    pending.append((exT, j, 2, uvp, av_start, av_stop, epi_m))

                def sitem_front(b, uvp, av_start, av_stop, epi_m, split=False):
                    """Solo diagonal item for odd slot b at key-group j=b."""
                    j = b
                    q_ap = qT[:, b * HGS:(b + 1) * HGS]
                    exT = expool.tile([128, 4, HGS], dt.bfloat16, tag="ex",
                                      name="exs")
                    ps4 = pscore.tile([128, 4, HGS], dt.float32, tag="sc",
                                      name="ps_s")
                    for r in range(4):
                        kb = 4 * j + r
                        nc.tensor.matmul(
                            ps4[:, r, :],
                            lhsT=kvT[0:64, kb * 128:(kb + 1) * 128],
                            rhs=q_ap, start=True, stop=True)
                    if split:
                        # halve exp so AV r=0,1 start sooner (tail latency)
                        nc.scalar.activation(
                            out=exT[:, 0:2, :], in_=ps4[:, 0:2, :],
                            func=mybir.ActivationFunctionType.Exp, scale=SCALE)
                        nc.vector.tensor_mul(exT[:, 0:2, :], exT[:, 0:2, :],
                                             tri_sb)
                        nc.scalar.activation(
                            out=exT[:, 2:4, :], in_=ps4[:, 2:4, :],
                            func=mybir.ActivationFunctionType.Exp, scale=SCALE)
                    else:
                        nc.scalar.activation(
                            out=exT, in_=ps4,
                            func=mybir.ActivationFunctionType.Exp, scale=SCALE)
                        nc.vector.tensor_mul(exT[:, 0:2, :], exT[:, 0:2, :],
                                             tri_sb)
                    nc.vector.tensor_scalar_mul(
                        exT[:, 2:4, :], exT[:, 2:4, :], dsel_sb[:, b:b + 1])
                    pending.append((exT, j, 1, uvp, av_start, av_stop, epi_m))

                def epilogue_slot(s, uvp, col0):
                    """One slot: PSUM->SBUF, 2x transpose, scale, one DMA."""
                    u_sb = epi.tile([H + 1, HGS], dt.float32, tag="usb")
                    nc.vector.tensor_copy(out=u_sb,
                                          in_=uvp[:, col0:col0 + HGS])
                    psts = []
                    for hh in range(2):
                        pst = paux.tile([128, H + 1], dt.float32, tag="a",
                                        name="pst_ep")
                        nc.tensor.transpose(
                            pst, u_sb[:, hh * 128:(hh + 1) * 128],
                            identity32[:, :])
                        psts.append(pst)
                    o_sb = epi.tile([128, 2, H], dt.float32, tag="o")
                    for hh, pst in enumerate(psts):
                        rec = epi.tile([128, 1], dt.float32, tag="rec")
                        nc.vector.reciprocal(rec, pst[:, H:H + 1])
                        nc.vector.tensor_scalar_mul(o_sb[:, hh, :], pst[:, 0:H],
                                                    rec)
                    nc.sync.dma_start(
                        out=out_e[:, s * 2 * H:(s + 1) * 2 * H]
                        .rearrange("p (hh h) -> p hh h", hh=2),
                        in_=o_sb)

                # ---- emission schedule ----
                # Input-DMA ring on SP in feed order.
                dma_wkq()
                qdma(0, 0, split=2)
                qdma(0, 1, split=2)
                qdma(1, 0, split=2)
                dma_wkv()
                dma_tri()
                fdma(0)
                qdma(1, 1)
                fdma(1)
                qdma(2, 0)
                fdma(2)
                qdma(2, 1)
                fdma(3)
                qdma(3, 0)
                qdma(3, 1)
                for j in range(4, NSLOT):
                    fdma(j)

                # PE warmup on identity while the first DMAs stream in.
                for i in range(N_WARM):
                    pw = paux.tile([128, 128], dt.float32, tag="a", name="warm")
                    nc.tensor.matmul(pw, lhsT=identity, rhs=identity,
                                     start=True, stop=True)

                # pair item streams, pipeline depth 3; epilogues auto-emit
                # right after their trigger AV flushes.
                uv_tiles = {}

                def F(p, j):
                    if p not in uv_tiles:
                        uv_tiles[p] = pu.tile([H + 1, 2 * HGS], dt.float32,
                                              tag="u", name=f"uv{p}")
                    epi_m = (2 * p, uv_tiles[p], 0) if j == 2 * p else None
                    pitem_front(p, j, uv_tiles[p], j == 0, j == 2 * p, epi_m)
                    while len(pending) > 2:
                        flush_av()

                def S(p, split=False):
                    epi_m = (2 * p + 1, uv_tiles[p], HGS)
                    sitem_front(2 * p + 1, uv_tiles[p], False, True, epi_m,
                                split=split)
                    while len(pending) > 2:
                        flush_av()

                def drain():
                    while pending:
                        flush_av()

                # emission order follows predicted data-arrival order (PE
                # executes its queue in order, so this IS the PE schedule);
                # v1own(5..7)/foreign(6..7) are deferred into the late
                # pure-item stretch, which is otherwise exp(ACT)-bound.
                own(0)
                own(1)
                own(2)
                v1own(0)
                v1own(1)
                v1own(2)
                foreign(0)
                F(0, 0)
                own(3)
                v1own(3)
                foreign(1)
                S(0)
                own(4)
                v1own(4)
                F(1, 0)
                F(1, 1)
                foreign(2)
                own(5)
                F(1, 2)
                foreign(3)
                own(6)
                S(1)
                own(7)
                F(2, 0)
                foreign(4)
                F(2, 1)
                foreign(5)
                F(2, 2)
                v1own(5)
                F(2, 3)
                F(2, 4)
                S(2)
                F(3, 0)
                foreign(6)
                F(3, 1)
                v1own(6)
                F(3, 2)
                foreign(7)
                F(3, 3)
                v1own(7)
                F(3, 4)
                F(3, 5)
                F(3, 6)
                S(3, split=True)
                drain()
    nc.compile()
    return nc


def _host_inputs(Wk, Wq, Wv):
    # device layout [p, et, m]: weight row et*128+p, col m
    wkv = _bf16(np.concatenate([Wk, Wv], axis=1)
                .reshape(ET, 128, 128).transpose(1, 0, 2).reshape(128, ET * 128))
    wkq = _bf16(np.concatenate([Wk, Wq], axis=1)
                .reshape(ET, 128, 128).transpose(1, 0, 2).reshape(128, ET * 128))
    rk = np.arange(HGS)[:, None]
    cq = np.arange(HGS)[None, :]
    tri = (rk <= cq).astype(np.float32)           # [256, 256] own triangle
    tri = _bf16(tri.reshape(2, 128, HGS).transpose(1, 0, 2).reshape(128, 2 * HGS))
    dsel = {}
    for half, hgs in ((0, HGS_A), (1, HGS_B)):
        d = np.array([[1.0 if hg % 2 == 1 else 0.0 for hg in hgs]] * 128,
                     dtype=np.float32)
        dsel[half] = np.ascontiguousarray(d)
    return wkv, wkq, tri, dsel


def kernel(x, Wk, Wq, Wv):
    from concourse.bass_utils import run_bass_kernel_spmd

    x = np.asarray(x, dtype=np.float32)
    Wk = np.asarray(Wk, dtype=np.float32)
    Wq = np.asarray(Wq, dtype=np.float32)
    Wv = np.asarray(Wv, dtype=np.float32)

    if "nc" not in _cache:
        _cache["nc"] = _build_graph()
    nc = _cache["nc"]

    wkv, wkq, tri, dsel = _host_inputs(Wk, Wq, Wv)

    in_maps = []
    core_meta = []
    for b in range(B):
        xTb = _bf16(x[b].T)  # [E, T]
        for half, hgs in enumerate([HGS_A, HGS_B]):
            other = [HGS_A, HGS_B][1 - half]
            xp = np.concatenate(
                [xTb[:, hg * HGS:(hg + 1) * HGS] for hg in list(hgs) + other],
                axis=1)
            in_maps.append({
                "xT": np.ascontiguousarray(xp),
                "wkv": wkv,
                "wkq": wkq,
                "tri": tri,
                "dsel": dsel[half],
            })
            core_meta.append((b, hgs))

    res = run_bass_kernel_spmd(nc, in_maps, core_ids=list(range(8)),
                               **_cache.get("run_kwargs", {}))
    _cache["last_result"] = res

    full = np.zeros((B, T, H), dtype=np.float32)
    for core, (b, hgs) in enumerate(core_meta):
        o = res.results[core]["out"]  # [128, NSLOT*2*H] partition-major
        o = o.reshape(128, NSLOT, 2, H).transpose(1, 2, 0, 3).reshape(NQ, H)
        for s, hg in enumerate(hgs):
            full[b, hg * HGS:(hg + 1) * HGS, :] = o[s * HGS:(s + 1) * HGS, :]
    return full


# revision 52
# speedup vs baseline: 1.2514x; 1.0194x over previous
"""Causal single-head attention (B=4, T=4096, E=1024, H=64) on 8 TRN2 cores.

Sharding: 2 cores per batch; no collectives (host shards, device computes,
host gathers). Queries are assigned to cores in 256-row half-groups with the
fold pattern {0,3}/{1,2} (mod 4), which makes both cores' causal work-lists
IDENTICAL: 8 query slots with key-group trip counts exactly (1..8), so one
SPMD graph serves all cores; all per-core variation (which queries, causal
mask content, key order) lives in host-prepared input data.

Host prep (layout-only, no FLOPs): x[b]^T cast to bf16 with columns permuted
to [owned half-groups in slot order | partner half-groups in the other
core's slot order]. Slot j's 256 queries are exactly the own half of
key-group j, so one fused [Wk|Wq] projection pass over the own columns
yields both K^T and Q^T; V1 for own tokens is computed directly
(lhsT=x-block, rhs=Wv) with full 128-partition output. Foreign columns get
a [Wk|Wv] pass + PE transposes for V1. The diagonal causal mask is a single
shared 256x256 triangle (identical for every slot and core) plus a per-slot
0/1 parity scalar for the foreign half -- tiny inputs instead of per-slot
masks.

Device (bf16 compute, f32 PSUM): items (pair p, key-group j) stream
pair-major; each pair's attn@V accumulates IN PSUM across its whole item
stream (matmul start/stop flags span items), so there are no per-item
accumulate ops at all. exp on ACT with scale=E^-0.5 folded in; V1 carries a
ones column so the softmax denominator falls out of the attn@V matmul.
Epilogue per slot: PSUM->SBUF copy, PE-transpose, reciprocal * scale, one
partition-major DMA. Input DMAs ride SP in arrival==consumption order
(weights pre-tiled host-side to dodge the sub-512B DMA penalty); PE warms up
on a memset tile while the first DMAs land. Emission order is hand-scheduled
against the DMA arrival timeline since each engine executes its queue
in order.
"""
import numpy as np
import ml_dtypes

B, T, E, H = 4, 4096, 1024, 64
HGS = 256         # queries per slot (half-group size)
KG = 512          # keys per key-group
NSLOT = 8
NQ = NSLOT * HGS  # 2048 owned queries per core
ET = E // 128     # 8 E-tiles
NKB = T // 128    # 32 key blocks
SCALE = float(E) ** -0.5
N_WARM = 26       # PE warmup matmuls (N=128) while first DMAs land

HGS_A = [0, 3, 4, 7, 8, 11, 12, 15]   # core half 0: needs 1..8 in slot order
HGS_B = [1, 2, 5, 6, 9, 10, 13, 14]   # core half 1: needs 1..8 in slot order

_cache = {}


def _bf16(a):
    return np.ascontiguousarray(a.astype(ml_dtypes.bfloat16))


def _build_graph():
    import concourse.mybir as mybir
    import concourse.tile as tile
    from concourse import bacc
    from concourse.masks import make_identity

    dt = mybir.dt
    nc = bacc.Bacc(None, target_bir_lowering=False)
    xT_e = nc.declare_dram_parameter("xT", [E, T], dt.bfloat16, isOutput=False)
    wkv_e = nc.declare_dram_parameter("wkv", [128, ET * 128], dt.bfloat16,
                                      isOutput=False)
    wkq_e = nc.declare_dram_parameter("wkq", [128, ET * 128], dt.bfloat16,
                                      isOutput=False)
    tri_e = nc.declare_dram_parameter("tri", [128, 2 * HGS], dt.bfloat16,
                                      isOutput=False)
    dsel_e = nc.declare_dram_parameter("dsel", [128, NSLOT], dt.float32,
                                       isOutput=False)
    out_e = nc.declare_dram_parameter("out", [128, NSLOT * 2 * H], dt.float32,
                                      isOutput=True)

    xT_r = xT_e.rearrange("(et p) t -> p et t", p=128)

    with tile.TileContext(nc) as tc:
        with (
            tc.tile_pool(name="singles", bufs=1) as singles,
            tc.tile_pool(name="persist", bufs=1) as persist,
        ):
            identity = singles.tile([128, 128], dt.bfloat16)
            make_identity(nc, identity)
            identity32 = singles.tile([H + 1, H + 1], dt.float32)
            make_identity(nc, identity32)
            wkv_sb = singles.tile([128, ET, 128], dt.bfloat16)
            wkq_sb = singles.tile([128, ET, 128], dt.bfloat16)
            tri_sb = singles.tile([128, 2, HGS], dt.bfloat16)
            dsel_sb = singles.tile([128, NSLOT], dt.float32)

            # persistent activations
            kvT = persist.tile([128, T], dt.bfloat16)    # 0:64 K^T, 64:128 V^T(frn)
            qT = persist.tile([64, NQ], dt.bfloat16)
            v1 = persist.tile([128, NKB, H + 1], dt.bfloat16)
            # per pair g: columns [own_2g | foreign_2g | own_2g+1 | foreign_2g+1]
            xq_tiles = [persist.tile([128, ET, 4, HGS], dt.bfloat16,
                                     name=f"xq{g}") for g in range(4)]

            nc.vector.memset(v1[:, :, H], 1.0)  # denominator ones column

            with (
                tc.tile_pool(name="pscore", bufs=2, space="PSUM") as pscore,
                tc.tile_pool(name="paux", bufs=2, space="PSUM") as paux,
                tc.tile_pool(name="pu", bufs=2, space="PSUM") as pu,
                tc.tile_pool(name="ex", bufs=4) as expool,
                tc.tile_pool(name="epi", bufs=4) as epi,
            ):
                # ---- DMA issue helpers (all inputs on SP, feed order) ----
                def dma_wkq():
                    nc.sync.dma_start(
                        out=wkq_sb,
                        in_=wkq_e.rearrange("p (et m) -> p et m", et=ET))

                def dma_wkv():
                    nc.sync.dma_start(
                        out=wkv_sb, in_=wkv_e.rearrange("p (et m) -> p et m", et=ET))

                def dma_tri():
                    nc.sync.dma_start(out=tri_sb,
                                      in_=tri_e.rearrange("p (r c) -> p r c", r=2))
                    nc.sync.dma_start(out=dsel_sb, in_=dsel_e[:, :])

                def qdma(g, two, split=1):
                    # own half for key-group 2g+two -> c-slot 2*two
                    step = ET // split
                    for h in range(split):
                        nc.sync.dma_start(
                            out=xq_tiles[g][:, h * step:(h + 1) * step, 2 * two, :],
                            in_=xT_r[:, h * step:(h + 1) * step,
                                     g * KG + two * HGS:g * KG + (two + 1) * HGS])

                def fdma(j):
                    # foreign half for key-group j -> c-slot 2*(j%2)+1
                    nc.sync.dma_start(
                        out=xq_tiles[j // 2][:, :, 2 * (j % 2) + 1, :],
                        in_=xT_r[:, :, NQ + j * HGS:NQ + (j + 1) * HGS])

                # ---- projection passes ----
                def own(j):
                    """[Wk|Wq] over own cols of key-group j: K^T own half +
                    Q^T of slot j (slot j's queries ARE its own keys).
                    Early groups' K copy rides idle ACT so K and Q copies
                    run in parallel (scores wait on both)."""
                    xo = xq_tiles[j // 2][:, :, 2 * (j % 2), :]
                    ps = paux.tile([128, HGS], dt.float32, tag="a")
                    for et in range(ET):
                        nc.tensor.matmul(ps, lhsT=wkq_sb[:, et, :],
                                         rhs=xo[:, et, :],
                                         start=(et == 0), stop=(et == ET - 1))
                    if j <= 3:
                        nc.scalar.copy(out=kvT[0:64, j * KG:j * KG + HGS],
                                       in_=ps[0:64, :])
                    else:
                        nc.vector.tensor_copy(
                            out=kvT[0:64, j * KG:j * KG + HGS], in_=ps[0:64, :])
                    nc.vector.tensor_copy(out=qT[:, j * HGS:(j + 1) * HGS],
                                          in_=ps[64:128, :])

                def v1own(j):
                    """V1 for own tokens of key-group j, directly:
                    out[tok,H] = sum_et x_blk^T.T @ Wv_et (full-M, free=64)."""
                    xo = xq_tiles[j // 2][:, :, 2 * (j % 2), :]
                    for b in range(2):
                        psv = paux.tile([128, H], dt.float32, tag="a", name="psv")
                        for et in range(ET):
                            nc.tensor.matmul(
                                psv, lhsT=xo[:, et, b * 128:(b + 1) * 128],
                                rhs=wkv_sb[:, et, 64:128],
                                start=(et == 0), stop=(et == ET - 1))
                        nc.vector.tensor_copy(out=v1[:, 4 * j + b, 0:H], in_=psv)

                def foreign(j):
                    """[Wk|Wv] over foreign cols of key-group j, then PE
                    transposes of V^T into V1 blocks 4j+2, 4j+3."""
                    xf = xq_tiles[j // 2][:, :, 2 * (j % 2) + 1, :]
                    ps = paux.tile([128, HGS], dt.float32, tag="a")
                    for et in range(ET):
                        nc.tensor.matmul(ps, lhsT=wkv_sb[:, et, :],
                                         rhs=xf[:, et, :],
                                         start=(et == 0), stop=(et == ET - 1))
                    nc.vector.tensor_copy(
                        out=kvT[:, j * KG + HGS:(j + 1) * KG], in_=ps)
                    for b in range(2):
                        kb = 4 * j + 2 + b
                        pst = paux.tile([128, H], dt.bfloat16, tag="a",
                                       name="pst_vt")
                        nc.tensor.transpose(
                            pst, kvT[64:128, kb * 128:(kb + 1) * 128],
                            identity[64:128, 64:128])
                        nc.vector.tensor_copy(out=v1[:, kb, 0:H], in_=pst)

                # ---- attention items, pair-major with PSUM-resident acc ----
                # pending: (exT, j, width, uv, av_start, av_stop, epi)
                pending = []

                def flush_av():
                    exT, j, w, uv, av_start, av_stop, epi_m, diag = \
                        pending.pop(0)
                    uvs = uv if w == 2 else uv[:, HGS:2 * HGS]
                    nq = w * HGS
                    for r in range(4):
                        if diag and r == 1:
                            # exT[:, 1, 0:128] is zero (masked) -- skip it
                            nc.tensor.matmul(
                                uvs[:, 128:nq], lhsT=v1[:, 4 * j + 1, :],
                                rhs=exT[:, 1, 128:nq],
                                start=False, stop=False,
                                skip_group_check=True)
                            continue
                        nc.tensor.matmul(
                            uvs, lhsT=v1[:, 4 * j + r, :], rhs=exT[:, r, :],
                            start=(av_start and r == 0),
                            stop=(av_stop and r == 3),
                            skip_group_check=True)
                    if epi_m:
                        for m in epi_m:
                            epilogue_slot(*m)

                def pitem_front(p, j, uvp, av_start, av_stop, epi_m):
                    """Paired item: slots (2p, 2p+1), key-group j, N=512.
                    Diagonal-masked on slot 2p's half when j == 2p."""
                    a = 2 * p
                    q_ap = qT[:, a * HGS:(a + 2) * HGS]
                    exT = expool.tile([128, 4, 2 * HGS], dt.bfloat16, tag="ex")
                    for half in range(2):
                        psh = pscore.tile([128, 2, 2 * HGS], dt.float32,
                                          tag="sc", name="ps_h")
                        for rr in range(2):
                            kb = 4 * j + 2 * half + rr
                            if j == a and half == 0 and rr == 1:
                                # own kb1 vs q 0:128 is fully causal-masked;
                                # skip it (exp of the stale corner is zeroed
                                # by the triangle mask)
                                nc.tensor.matmul(
                                    psh[:, 1, 128:2 * HGS],
                                    lhsT=kvT[0:64, kb * 128:(kb + 1) * 128],
                                    rhs=q_ap[:, 128:2 * HGS],
                                    start=True, stop=True)
                                continue
                            nc.tensor.matmul(
                                psh[:, rr, :],
                                lhsT=kvT[0:64, kb * 128:(kb + 1) * 128],
                                rhs=q_ap, start=True, stop=True)
                        nc.scalar.activation(
                            out=exT[:, 2 * half:2 * half + 2, :], in_=psh,
                            func=mybir.ActivationFunctionType.Exp, scale=SCALE)
                        if j == a and half == 0:
                            nc.vector.tensor_mul(
                                exT[:, 0:2, 0:HGS], exT[:, 0:2, 0:HGS], tri_sb)
                    if j == a:  # diagonal foreign half of slot 2p
                        nc.vector.tensor_scalar_mul(
                            exT[:, 2:4, 0:HGS], exT[:, 2:4, 0:HGS],
                            dsel_sb[:, a:a + 1])
                    pending.append((exT, j, 2, uvp, av_start, av_stop,
                                    epi_m, j == a))

                def sitem_front(b, uvp, av_start, av_stop, epi_m, split=False):
                    """Solo diagonal item for odd slot b at key-group j=b."""
                    j = b
                    q_ap = qT[:, b * HGS:(b + 1) * HGS]
                    exT = expool.tile([128, 4, HGS], dt.bfloat16, tag="ex",
                                      name="exs")
                    ps4 = pscore.tile([128, 4, HGS], dt.float32, tag="sc",
                                      name="ps_s")
                    for r in range(4):
                        kb = 4 * j + r
                        if r == 1:
                            nc.tensor.matmul(
                                ps4[:, 1, 128:HGS],
                                lhsT=kvT[0:64, kb * 128:(kb + 1) * 128],
                                rhs=q_ap[:, 128:HGS], start=True, stop=True)
                            continue
                        nc.tensor.matmul(
                            ps4[:, r, :],
                            lhsT=kvT[0:64, kb * 128:(kb + 1) * 128],
                            rhs=q_ap, start=True, stop=True)
                    if split:
                        # halve exp so AV r=0,1 start sooner (tail latency)
                        nc.scalar.activation(
                            out=exT[:, 0:2, :], in_=ps4[:, 0:2, :],
                            func=mybir.ActivationFunctionType.Exp, scale=SCALE)
                        nc.vector.tensor_mul(exT[:, 0:2, :], exT[:, 0:2, :],
                                             tri_sb)
                        nc.scalar.activation(
                            out=exT[:, 2:4, :], in_=ps4[:, 2:4, :],
                            func=mybir.ActivationFunctionType.Exp, scale=SCALE)
                    else:
                        nc.scalar.activation(
                            out=exT, in_=ps4,
                            func=mybir.ActivationFunctionType.Exp, scale=SCALE)
                        nc.vector.tensor_mul(exT[:, 0:2, :], exT[:, 0:2, :],
                                             tri_sb)
                    nc.vector.tensor_scalar_mul(
                        exT[:, 2:4, :], exT[:, 2:4, :], dsel_sb[:, b:b + 1])
                    pending.append((exT, j, 1, uvp, av_start, av_stop,
                                    epi_m, True))

                def epilogue_slot(s, uvp, col0):
                    """One slot: PSUM->SBUF, 2x transpose, scale, one DMA."""
                    u_sb = epi.tile([H + 1, HGS], dt.float32, tag="usb")
                    nc.vector.tensor_copy(out=u_sb,
                                          in_=uvp[:, col0:col0 + HGS])
                    psts = []
                    for hh in range(2):
                        pst = paux.tile([128, H + 1], dt.float32, tag="a",
                                        name="pst_ep")
                        nc.tensor.transpose(
                            pst, u_sb[:, hh * 128:(hh + 1) * 128],
                            identity32[:, :])
                        psts.append(pst)
                    o_sb = epi.tile([128, 2, H], dt.float32, tag="o")
                    for hh, pst in enumerate(psts):
                        rec = epi.tile([128, 1], dt.float32, tag="rec")
                        nc.vector.reciprocal(rec, pst[:, H:H + 1])
                        nc.vector.tensor_scalar_mul(o_sb[:, hh, :], pst[:, 0:H],
                                                    rec)
                    nc.sync.dma_start(
                        out=out_e[:, s * 2 * H:(s + 1) * 2 * H]
                        .rearrange("p (hh h) -> p hh h", hh=2),
                        in_=o_sb)

                # ---- emission schedule ----
                # Input-DMA ring on SP in feed order.
                dma_wkq()
                qdma(0, 0, split=2)
                qdma(0, 1, split=2)
                qdma(1, 0, split=2)
                dma_wkv()
                dma_tri()
                fdma(0)
                qdma(1, 1)
                fdma(1)
                qdma(2, 0)
                fdma(2)
                qdma(2, 1)
                fdma(3)
                qdma(3, 0)
                qdma(3, 1)
                for j in range(4, NSLOT):
                    fdma(j)

                # PE warmup on identity while the first DMAs stream in.
                for i in range(N_WARM):
                    pw = paux.tile([128, 128], dt.float32, tag="a", name="warm")
                    nc.tensor.matmul(pw, lhsT=identity, rhs=identity,
                                     start=True, stop=True)

                # scrub the two score-PSUM ring buffers once: diag items
                # skip a sub-block whose stale contents feed exp; first-ever
                # tenants are raw PSUM bits that could blow up exp.
                for _ in range(2):
                    scrub = pscore.tile([128, 2, 2 * HGS], dt.float32,
                                        tag="sc", name="scrub")
                    nc.vector.memset(scrub, 0.0)

                # pair item streams, pipeline depth 3; epilogues auto-emit
                # right after their trigger AV flushes.
                uv_tiles = {}

                epi_defer = {}  # (p, j) -> list of (slot, pair, col0)

                def F(p, j):
                    if p not in uv_tiles:
                        uv_tiles[p] = pu.tile([H + 1, 2 * HGS], dt.float32,
                                              tag="u", name=f"uv{p}")
                    if (p, j) in epi_defer:
                        epi_m = [(s, uv_tiles[pp], c)
                                 for s, pp, c in epi_defer[(p, j)]]
                    elif j == 2 * p and (p, -1) not in epi_defer:
                        epi_m = [(2 * p, uv_tiles[p], 0)]
                    else:
                        epi_m = None
                    pitem_front(p, j, uv_tiles[p], j == 0, j == 2 * p, epi_m)
                    while len(pending) > 2:
                        flush_av()

                def S(p, split=False):
                    if (p, -2) in epi_defer:
                        epi_m = None
                    else:
                        epi_m = [(2 * p + 1, uv_tiles[p], HGS)]
                    sitem_front(2 * p + 1, uv_tiles[p], False, True, epi_m,
                                split=split)
                    while len(pending) > 2:
                        flush_av()

                def drain():
                    while pending:
                        flush_av()

                # emission order follows predicted data-arrival order (PE
                # executes its queue in order, so this IS the PE schedule);
                # v1own(5..7)/foreign(6..7) are deferred into the late
                # pure-item stretch, which is otherwise exp(ACT)-bound.
                epi_defer[(2, -1)] = True   # suppress pair-2 auto even-epi
                epi_defer[(2, -2)] = True   # suppress pair-2 solo epi
                epi_defer[(3, 2)] = [(4, 2, 0)]
                epi_defer[(3, 3)] = [(5, 2, HGS)]

                own(0)
                own(1)
                own(2)
                v1own(0)
                v1own(1)
                v1own(2)
                foreign(0)
                F(0, 0)
                own(3)
                v1own(3)
                foreign(1)
                S(0)
                own(4)
                v1own(4)
                F(1, 0)
                foreign(2)
                F(1, 1)
                own(5)
                F(1, 2)
                foreign(3)
                own(6)
                S(1)
                own(7)
                F(2, 0)
                foreign(4)
                F(2, 1)
                foreign(5)
                F(2, 2)
                v1own(5)
                F(2, 3)
                F(2, 4)
                S(2)
                F(3, 0)
                foreign(6)
                F(3, 1)
                v1own(6)
                F(3, 2)
                foreign(7)
                F(3, 3)
                v1own(7)
                F(3, 4)
                F(3, 5)
                F(3, 6)
                S(3, split=True)
                drain()
    nc.compile()
    return nc


def _host_inputs(Wk, Wq, Wv):
    # device layout [p, et, m]: weight row et*128+p, col m
    wkv = _bf16(np.concatenate([Wk, Wv], axis=1)
                .reshape(ET, 128, 128).transpose(1, 0, 2).reshape(128, ET * 128))
    wkq = _bf16(np.concatenate([Wk, Wq], axis=1)
                .reshape(ET, 128, 128).transpose(1, 0, 2).reshape(128, ET * 128))
    rk = np.arange(HGS)[:, None]
    cq = np.arange(HGS)[None, :]
    tri = (rk <= cq).astype(np.float32)           # [256, 256] own triangle
    tri = _bf16(tri.reshape(2, 128, HGS).transpose(1, 0, 2).reshape(128, 2 * HGS))
    dsel = {}
    for half, hgs in ((0, HGS_A), (1, HGS_B)):
        d = np.array([[1.0 if hg % 2 == 1 else 0.0 for hg in hgs]] * 128,
                     dtype=np.float32)
        dsel[half] = np.ascontiguousarray(d)
    return wkv, wkq, tri, dsel


def kernel(x, Wk, Wq, Wv):
    from concourse.bass_utils import run_bass_kernel_spmd

    x = np.asarray(x, dtype=np.float32)
    Wk = np.asarray(Wk, dtype=np.float32)
    Wq = np.asarray(Wq, dtype=np.float32)
    Wv = np.asarray(Wv, dtype=np.float32)

    if "nc" not in _cache:
        _cache["nc"] = _build_graph()
    nc = _cache["nc"]

    wkv, wkq, tri, dsel = _host_inputs(Wk, Wq, Wv)

    in_maps = []
    core_meta = []
    for b in range(B):
        xTb = _bf16(x[b].T)  # [E, T]
        for half, hgs in enumerate([HGS_A, HGS_B]):
            other = [HGS_A, HGS_B][1 - half]
            xp = np.concatenate(
                [xTb[:, hg * HGS:(hg + 1) * HGS] for hg in list(hgs) + other],
                axis=1)
            in_maps.append({
                "xT": np.ascontiguousarray(xp),
                "wkv": wkv,
                "wkq": wkq,
                "tri": tri,
                "dsel": dsel[half],
            })
            core_meta.append((b, hgs))

    res = run_bass_kernel_spmd(nc, in_maps, core_ids=list(range(8)),
                               **_cache.get("run_kwargs", {}))
    _cache["last_result"] = res

    full = np.zeros((B, T, H), dtype=np.float32)
    for core, (b, hgs) in enumerate(core_meta):
        o = res.results[core]["out"]  # [128, NSLOT*2*H] partition-major
        o = o.reshape(128, NSLOT, 2, H).transpose(1, 2, 0, 3).reshape(NQ, H)
        for s, hg in enumerate(hgs):
            full[b, hg * HGS:(hg + 1) * HGS, :] = o[s * HGS:(s + 1) * HGS, :]
    return full


# revision 54
# speedup vs baseline: 1.2593x; 1.0063x over previous
"""Causal single-head attention (B=4, T=4096, E=1024, H=64) on 8 TRN2 cores.

Sharding: 2 cores per batch; no collectives (host shards, device computes,
host gathers). Queries are assigned to cores in 256-row half-groups with the
fold pattern {0,3}/{1,2} (mod 4), which makes both cores' causal work-lists
IDENTICAL: 8 query slots with key-group trip counts exactly (1..8), so one
SPMD graph serves all cores; all per-core variation (which queries, causal
mask content, key order) lives in host-prepared input data.

Host prep (layout-only, no FLOPs): x[b]^T cast to bf16 with columns permuted
to [owned half-groups in slot order | partner half-groups in the other
core's slot order]. Slot j's 256 queries are exactly the own half of
key-group j, so one fused [Wk|Wq] projection pass over the own columns
yields both K^T and Q^T; V1 for own tokens is computed directly
(lhsT=x-block, rhs=Wv) with full 128-partition output. Foreign columns get
a [Wk|Wv] pass + PE transposes for V1. The diagonal causal mask is a single
shared 256x256 triangle (identical for every slot and core) plus a per-slot
0/1 parity scalar for the foreign half -- tiny inputs instead of per-slot
masks.

Device (bf16 compute, f32 PSUM): items (pair p, key-group j) stream
pair-major; each pair's attn@V accumulates IN PSUM across its whole item
stream (matmul start/stop flags span items), so there are no per-item
accumulate ops at all. exp on ACT with scale=E^-0.5 folded in; V1 carries a
ones column so the softmax denominator falls out of the attn@V matmul.
Epilogue per slot: PSUM->SBUF copy, PE-transpose, reciprocal * scale, one
partition-major DMA. Input DMAs ride SP in arrival==consumption order
(weights pre-tiled host-side to dodge the sub-512B DMA penalty); PE warms up
on a memset tile while the first DMAs land. Emission order is hand-scheduled
against the DMA arrival timeline since each engine executes its queue
in order.
"""
import numpy as np
import ml_dtypes

B, T, E, H = 4, 4096, 1024, 64
HGS = 256         # queries per slot (half-group size)
KG = 512          # keys per key-group
NSLOT = 8
NQ = NSLOT * HGS  # 2048 owned queries per core
ET = E // 128     # 8 E-tiles
NKB = T // 128    # 32 key blocks
SCALE = float(E) ** -0.5
N_WARM = 26       # PE warmup matmuls (N=128) while first DMAs land

HGS_A = [0, 3, 4, 7, 8, 11, 12, 15]   # core half 0: needs 1..8 in slot order
HGS_B = [1, 2, 5, 6, 9, 10, 13, 14]   # core half 1: needs 1..8 in slot order

_cache = {}


def _bf16(a):
    return np.ascontiguousarray(a.astype(ml_dtypes.bfloat16))


def _build_graph():
    import concourse.mybir as mybir
    import concourse.tile as tile
    from concourse import bacc
    from concourse.masks import make_identity

    dt = mybir.dt
    nc = bacc.Bacc(None, target_bir_lowering=False)
    xT_e = nc.declare_dram_parameter("xT", [E, T], dt.bfloat16, isOutput=False)
    wkv_e = nc.declare_dram_parameter("wkv", [128, ET * 128], dt.bfloat16,
                                      isOutput=False)
    wkq_e = nc.declare_dram_parameter("wkq", [128, ET * 128], dt.bfloat16,
                                      isOutput=False)
    tri_e = nc.declare_dram_parameter("tri", [128, 2 * HGS], dt.bfloat16,
                                      isOutput=False)
    dsel_e = nc.declare_dram_parameter("dsel", [128, NSLOT], dt.float32,
                                       isOutput=False)
    out_e = nc.declare_dram_parameter("out", [128, NSLOT * 2 * H], dt.float32,
                                      isOutput=True)

    xT_r = xT_e.rearrange("(et p) t -> p et t", p=128)

    with tile.TileContext(nc) as tc:
        with (
            tc.tile_pool(name="singles", bufs=1) as singles,
            tc.tile_pool(name="persist", bufs=1) as persist,
        ):
            identity = singles.tile([128, 128], dt.bfloat16)
            make_identity(nc, identity)
            identity32 = singles.tile([H + 1, H + 1], dt.float32)
            make_identity(nc, identity32)
            wkv_sb = singles.tile([128, ET, 128], dt.bfloat16)
            wkq_sb = singles.tile([128, ET, 128], dt.bfloat16)
            tri_sb = singles.tile([128, 2, HGS], dt.bfloat16)
            dsel_sb = singles.tile([128, NSLOT], dt.float32)

            # persistent activations
            kvT = persist.tile([128, T], dt.bfloat16)    # 0:64 K^T, 64:128 V^T(frn)
            qT = persist.tile([64, NQ], dt.bfloat16)
            v1 = persist.tile([128, NKB, H + 1], dt.bfloat16)
            # per pair g: columns [own_2g | foreign_2g | own_2g+1 | foreign_2g+1]
            xq_tiles = [persist.tile([128, ET, 4, HGS], dt.bfloat16,
                                     name=f"xq{g}") for g in range(4)]

            nc.vector.memset(v1[:, :, H], 1.0)  # denominator ones column

            with (
                tc.tile_pool(name="pscore", bufs=2, space="PSUM") as pscore,
                tc.tile_pool(name="paux", bufs=2, space="PSUM") as paux,
                tc.tile_pool(name="pu", bufs=2, space="PSUM") as pu,
                tc.tile_pool(name="ex", bufs=4) as expool,
                tc.tile_pool(name="epi", bufs=4) as epi,
            ):
                # ---- DMA issue helpers (all inputs on SP, feed order) ----
                def dma_wkq():
                    nc.sync.dma_start(
                        out=wkq_sb,
                        in_=wkq_e.rearrange("p (et m) -> p et m", et=ET))

                def dma_wkv():
                    wr = wkv_e.rearrange("p (et m) -> p et m", et=ET)
                    nc.sync.dma_start(out=wkv_sb[:, 0:4, :], in_=wr[:, 0:4, :])
                    nc.sync.dma_start(out=wkv_sb[:, 4:8, :], in_=wr[:, 4:8, :])

                def dma_tri():
                    nc.sync.dma_start(out=tri_sb,
                                      in_=tri_e.rearrange("p (r c) -> p r c", r=2))
                    nc.sync.dma_start(out=dsel_sb, in_=dsel_e[:, :])

                def qdma(g, two, split=1):
                    # own half for key-group 2g+two -> c-slot 2*two
                    step = ET // split
                    for h in range(split):
                        nc.sync.dma_start(
                            out=xq_tiles[g][:, h * step:(h + 1) * step, 2 * two, :],
                            in_=xT_r[:, h * step:(h + 1) * step,
                                     g * KG + two * HGS:g * KG + (two + 1) * HGS])

                def fdma(j):
                    # foreign half for key-group j -> c-slot 2*(j%2)+1
                    nc.sync.dma_start(
                        out=xq_tiles[j // 2][:, :, 2 * (j % 2) + 1, :],
                        in_=xT_r[:, :, NQ + j * HGS:NQ + (j + 1) * HGS])

                # ---- projection passes ----
                def own(j):
                    """[Wk|Wq] over own cols of key-group j: K^T own half +
                    Q^T of slot j (slot j's queries ARE its own keys).
                    Early groups' K copy rides idle ACT so K and Q copies
                    run in parallel (scores wait on both)."""
                    xo = xq_tiles[j // 2][:, :, 2 * (j % 2), :]
                    ps = paux.tile([128, HGS], dt.float32, tag="a")
                    for et in range(ET):
                        nc.tensor.matmul(ps, lhsT=wkq_sb[:, et, :],
                                         rhs=xo[:, et, :],
                                         start=(et == 0), stop=(et == ET - 1))
                    if j <= 3:
                        nc.scalar.copy(out=kvT[0:64, j * KG:j * KG + HGS],
                                       in_=ps[0:64, :])
                    else:
                        nc.vector.tensor_copy(
                            out=kvT[0:64, j * KG:j * KG + HGS], in_=ps[0:64, :])
                    nc.vector.tensor_copy(out=qT[:, j * HGS:(j + 1) * HGS],
                                          in_=ps[64:128, :])

                def v1own(j):
                    """V1 for own tokens of key-group j, directly:
                    out[tok,H] = sum_et x_blk^T.T @ Wv_et (full-M, free=64)."""
                    xo = xq_tiles[j // 2][:, :, 2 * (j % 2), :]
                    for b in range(2):
                        psv = paux.tile([128, H], dt.float32, tag="a", name="psv")
                        for et in range(ET):
                            nc.tensor.matmul(
                                psv, lhsT=xo[:, et, b * 128:(b + 1) * 128],
                                rhs=wkv_sb[:, et, 64:128],
                                start=(et == 0), stop=(et == ET - 1))
                        nc.vector.tensor_copy(out=v1[:, 4 * j + b, 0:H], in_=psv)

                def foreign(j):
                    """[Wk|Wv] over foreign cols of key-group j, then PE
                    transposes of V^T into V1 blocks 4j+2, 4j+3."""
                    xf = xq_tiles[j // 2][:, :, 2 * (j % 2) + 1, :]
                    ps = paux.tile([128, HGS], dt.float32, tag="a")
                    for et in range(ET):
                        nc.tensor.matmul(ps, lhsT=wkv_sb[:, et, :],
                                         rhs=xf[:, et, :],
                                         start=(et == 0), stop=(et == ET - 1))
                    nc.vector.tensor_copy(
                        out=kvT[:, j * KG + HGS:(j + 1) * KG], in_=ps)
                    for b in range(2):
                        kb = 4 * j + 2 + b
                        pst = paux.tile([128, H], dt.bfloat16, tag="a",
                                       name="pst_vt")
                        nc.tensor.transpose(
                            pst, kvT[64:128, kb * 128:(kb + 1) * 128],
                            identity[64:128, 64:128])
                        nc.vector.tensor_copy(out=v1[:, kb, 0:H], in_=pst)

                # ---- attention items, pair-major with PSUM-resident acc ----
                # pending: (exT, j, width, uv, av_start, av_stop, epi)
                pending = []

                def flush_av():
                    exT, j, w, uv, av_start, av_stop, epi_m, diag = \
                        pending.pop(0)
                    uvs = uv if w == 2 else uv[:, HGS:2 * HGS]
                    nq = w * HGS
                    for r in range(4):
                        if diag and r == 1:
                            # exT[:, 1, 0:128] is zero (masked) -- skip it
                            nc.tensor.matmul(
                                uvs[:, 128:nq], lhsT=v1[:, 4 * j + 1, :],
                                rhs=exT[:, 1, 128:nq],
                                start=False, stop=False,
                                skip_group_check=True)
                            continue
                        nc.tensor.matmul(
                            uvs, lhsT=v1[:, 4 * j + r, :], rhs=exT[:, r, :],
                            start=(av_start and r == 0),
                            stop=(av_stop and r == 3),
                            skip_group_check=True)
                    if epi_m:
                        for m in epi_m:
                            epilogue_slot(*m)

                def pitem_front(p, j, uvp, av_start, av_stop, epi_m):
                    """Paired item: slots (2p, 2p+1), key-group j, N=512.
                    Diagonal-masked on slot 2p's half when j == 2p."""
                    a = 2 * p
                    q_ap = qT[:, a * HGS:(a + 2) * HGS]
                    exT = expool.tile([128, 4, 2 * HGS], dt.bfloat16, tag="ex")
                    for half in range(2):
                        psh = pscore.tile([128, 2, 2 * HGS], dt.float32,
                                          tag="sc", name="ps_h")
                        for rr in range(2):
                            kb = 4 * j + 2 * half + rr
                            if j == a and half == 0 and rr == 1:
                                # own kb1 vs q 0:128 is fully causal-masked;
                                # skip it (exp of the stale corner is zeroed
                                # by the triangle mask)
                                nc.tensor.matmul(
                                    psh[:, 1, 128:2 * HGS],
                                    lhsT=kvT[0:64, kb * 128:(kb + 1) * 128],
                                    rhs=q_ap[:, 128:2 * HGS],
                                    start=True, stop=True)
                                continue
                            nc.tensor.matmul(
                                psh[:, rr, :],
                                lhsT=kvT[0:64, kb * 128:(kb + 1) * 128],
                                rhs=q_ap, start=True, stop=True)
                        nc.scalar.activation(
                            out=exT[:, 2 * half:2 * half + 2, :], in_=psh,
                            func=mybir.ActivationFunctionType.Exp, scale=SCALE)
                        if j == a and half == 0:
                            nc.vector.tensor_mul(
                                exT[:, 0:2, 0:HGS], exT[:, 0:2, 0:HGS], tri_sb)
                    if j == a:  # diagonal foreign half of slot 2p
                        nc.vector.tensor_scalar_mul(
                            exT[:, 2:4, 0:HGS], exT[:, 2:4, 0:HGS],
                            dsel_sb[:, a:a + 1])
                    pending.append((exT, j, 2, uvp, av_start, av_stop,
                                    epi_m, j == a))

                def sitem_front(b, uvp, av_start, av_stop, epi_m, split=False):
                    """Solo diagonal item for odd slot b at key-group j=b."""
                    j = b
                    q_ap = qT[:, b * HGS:(b + 1) * HGS]
                    exT = expool.tile([128, 4, HGS], dt.bfloat16, tag="ex",
                                      name="exs")
                    ps4 = pscore.tile([128, 4, HGS], dt.float32, tag="sc",
                                      name="ps_s")
                    for r in range(4):
                        kb = 4 * j + r
                        if r == 1:
                            nc.tensor.matmul(
                                ps4[:, 1, 128:HGS],
                                lhsT=kvT[0:64, kb * 128:(kb + 1) * 128],
                                rhs=q_ap[:, 128:HGS], start=True, stop=True)
                            continue
                        nc.tensor.matmul(
                            ps4[:, r, :],
                            lhsT=kvT[0:64, kb * 128:(kb + 1) * 128],
                            rhs=q_ap, start=True, stop=True)
                    if split:
                        # halve exp so AV r=0,1 start sooner (tail latency)
                        nc.scalar.activation(
                            out=exT[:, 0:2, :], in_=ps4[:, 0:2, :],
                            func=mybir.ActivationFunctionType.Exp, scale=SCALE)
                        nc.vector.tensor_mul(exT[:, 0:2, :], exT[:, 0:2, :],
                                             tri_sb)
                        nc.scalar.activation(
                            out=exT[:, 2:4, :], in_=ps4[:, 2:4, :],
                            func=mybir.ActivationFunctionType.Exp, scale=SCALE)
                    else:
                        nc.scalar.activation(
                            out=exT, in_=ps4,
                            func=mybir.ActivationFunctionType.Exp, scale=SCALE)
                        nc.vector.tensor_mul(exT[:, 0:2, :], exT[:, 0:2, :],
                                             tri_sb)
                    nc.vector.tensor_scalar_mul(
                        exT[:, 2:4, :], exT[:, 2:4, :], dsel_sb[:, b:b + 1])
                    pending.append((exT, j, 1, uvp, av_start, av_stop,
                                    epi_m, True))

                def epilogue_slot(s, uvp, col0):
                    """One slot: PSUM->SBUF, 2x transpose, scale, one DMA."""
                    u_sb = epi.tile([H + 1, HGS], dt.float32, tag="usb")
                    nc.vector.tensor_copy(out=u_sb,
                                          in_=uvp[:, col0:col0 + HGS])
                    psts = []
                    for hh in range(2):
                        pst = paux.tile([128, H + 1], dt.float32, tag="a",
                                        name="pst_ep")
                        nc.tensor.transpose(
                            pst, u_sb[:, hh * 128:(hh + 1) * 128],
                            identity32[:, :])
                        psts.append(pst)
                    o_sb = epi.tile([128, 2, H], dt.float32, tag="o")
                    for hh, pst in enumerate(psts):
                        rec = epi.tile([128, 1], dt.float32, tag="rec")
                        nc.vector.reciprocal(rec, pst[:, H:H + 1])
                        nc.vector.tensor_scalar_mul(o_sb[:, hh, :], pst[:, 0:H],
                                                    rec)
                    nc.sync.dma_start(
                        out=out_e[:, s * 2 * H:(s + 1) * 2 * H]
                        .rearrange("p (hh h) -> p hh h", hh=2),
                        in_=o_sb)

                # ---- emission schedule ----
                # Input-DMA ring on SP in feed order.
                dma_wkq()
                qdma(0, 0, split=2)
                qdma(0, 1, split=2)
                qdma(1, 0, split=2)
                dma_wkv()
                dma_tri()
                fdma(0)
                qdma(1, 1)
                fdma(1)
                qdma(2, 0)
                fdma(2)
                qdma(2, 1)
                fdma(3)
                qdma(3, 0)
                qdma(3, 1)
                for j in range(4, NSLOT):
                    fdma(j)

                # PE warmup on identity while the first DMAs stream in.
                for i in range(N_WARM):
                    pw = paux.tile([128, 128], dt.float32, tag="a", name="warm")
                    nc.tensor.matmul(pw, lhsT=identity, rhs=identity,
                                     start=True, stop=True)

                # scrub the two score-PSUM ring buffers once: diag items
                # skip a sub-block whose stale contents feed exp; first-ever
                # tenants are raw PSUM bits that could blow up exp.
                for _ in range(2):
                    scrub = pscore.tile([128, 2, 2 * HGS], dt.float32,
                                        tag="sc", name="scrub")
                    nc.vector.memset(scrub, 0.0)

                # pair item streams, pipeline depth 3; epilogues auto-emit
                # right after their trigger AV flushes.
                uv_tiles = {}

                epi_defer = {}  # (p, j) -> list of (slot, pair, col0)

                def F(p, j):
                    if p not in uv_tiles:
                        uv_tiles[p] = pu.tile([H + 1, 2 * HGS], dt.float32,
                                              tag="u", name=f"uv{p}")
                    if (p, j) in epi_defer:
                        epi_m = [(s, uv_tiles[pp], c)
                                 for s, pp, c in epi_defer[(p, j)]]
                    elif j == 2 * p and (p, -1) not in epi_defer:
                        epi_m = [(2 * p, uv_tiles[p], 0)]
                    else:
                        epi_m = None
                    pitem_front(p, j, uv_tiles[p], j == 0, j == 2 * p, epi_m)
                    while len(pending) > 2:
                        flush_av()

                def S(p, split=False):
                    if (p, -2) in epi_defer:
                        epi_m = None
                    else:
                        epi_m = [(2 * p + 1, uv_tiles[p], HGS)]
                    sitem_front(2 * p + 1, uv_tiles[p], False, True, epi_m,
                                split=split)
                    while len(pending) > 2:
                        flush_av()

                def drain():
                    while pending:
                        flush_av()

                # emission order follows predicted data-arrival order (PE
                # executes its queue in order, so this IS the PE schedule);
                # v1own(5..7)/foreign(6..7) are deferred into the late
                # pure-item stretch, which is otherwise exp(ACT)-bound.
                epi_defer[(2, -1)] = True   # suppress pair-2 auto even-epi
                epi_defer[(2, -2)] = True   # suppress pair-2 solo epi
                epi_defer[(3, 2)] = [(4, 2, 0)]
                epi_defer[(3, 3)] = [(5, 2, HGS)]

                own(0)
                own(1)
                own(2)
                v1own(0)
                v1own(1)
                v1own(2)
                foreign(0)
                F(0, 0)
                own(3)
                v1own(3)
                foreign(1)
                S(0)
                own(4)
                v1own(4)
                F(1, 0)
                foreign(2)
                F(1, 1)
                own(5)
                F(1, 2)
                foreign(3)
                own(6)
                S(1)
                own(7)
                F(2, 0)
                foreign(4)
                F(2, 1)
                foreign(5)
                F(2, 2)
                v1own(5)
                F(2, 3)
                F(2, 4)
                S(2)
                F(3, 0)
                foreign(6)
                F(3, 1)
                F(3, 2)
                v1own(6)
                F(3, 3)
                foreign(7)
                F(3, 4)
                v1own(7)
                F(3, 5)
                F(3, 6)
                S(3, split=True)
                drain()
    nc.compile()
    return nc


def _host_inputs(Wk, Wq, Wv):
    # device layout [p, et, m]: weight row et*128+p, col m
    wkv = _bf16(np.concatenate([Wk, Wv], axis=1)
                .reshape(ET, 128, 128).transpose(1, 0, 2).reshape(128, ET * 128))
    wkq = _bf16(np.concatenate([Wk, Wq], axis=1)
                .reshape(ET, 128, 128).transpose(1, 0, 2).reshape(128, ET * 128))
    rk = np.arange(HGS)[:, None]
    cq = np.arange(HGS)[None, :]
    tri = (rk <= cq).astype(np.float32)           # [256, 256] own triangle
    tri = _bf16(tri.reshape(2, 128, HGS).transpose(1, 0, 2).reshape(128, 2 * HGS))
    dsel = {}
    for half, hgs in ((0, HGS_A), (1, HGS_B)):
        d = np.array([[1.0 if hg % 2 == 1 else 0.0 for hg in hgs]] * 128,
                     dtype=np.float32)
        dsel[half] = np.ascontiguousarray(d)
    return wkv, wkq, tri, dsel


def kernel(x, Wk, Wq, Wv):
    from concourse.bass_utils import run_bass_kernel_spmd

    x = np.asarray(x, dtype=np.float32)
    Wk = np.asarray(Wk, dtype=np.float32)
    Wq = np.asarray(Wq, dtype=np.float32)
    Wv = np.asarray(Wv, dtype=np.float32)

    if "nc" not in _cache:
        _cache["nc"] = _build_graph()
    nc = _cache["nc"]

    wkv, wkq, tri, dsel = _host_inputs(Wk, Wq, Wv)

    in_maps = []
    core_meta = []
    for b in range(B):
        xTb = _bf16(x[b].T)  # [E, T]
        for half, hgs in enumerate([HGS_A, HGS_B]):
            other = [HGS_A, HGS_B][1 - half]
            xp = np.concatenate(
                [xTb[:, hg * HGS:(hg + 1) * HGS] for hg in list(hgs) + other],
                axis=1)
            in_maps.append({
                "xT": np.ascontiguousarray(xp),
                "wkv": wkv,
                "wkq": wkq,
                "tri": tri,
                "dsel": dsel[half],
            })
            core_meta.append((b, hgs))

    res = run_bass_kernel_spmd(nc, in_maps, core_ids=list(range(8)),
                               **_cache.get("run_kwargs", {}))
    _cache["last_result"] = res

    full = np.zeros((B, T, H), dtype=np.float32)
    for core, (b, hgs) in enumerate(core_meta):
        o = res.results[core]["out"]  # [128, NSLOT*2*H] partition-major
        o = o.reshape(128, NSLOT, 2, H).transpose(1, 2, 0, 3).reshape(NQ, H)
        for s, hg in enumerate(hgs):
            full[b, hg * HGS:(hg + 1) * HGS, :] = o[s * HGS:(s + 1) * HGS, :]
    return full
